# revision 25
# baseline (speedup 1.0000x reference)
"""Multi-head attention TRN2 Bass kernel (v2).

Problem: B=4, N=2048, D=E=512, 8 heads (ch=64).
out = softmax((x_q Wq + bq)(x_k Wk + bk)^T / 8) (x_v Wv + bv), per head.

Sharding (8 cores): core c handles batch b = c//2 and head-group g = c%2
(4 heads = 256 E-columns). Each core is fully independent (no collectives).

v2 changes over the original ACT-paced design:
  - Pass = (head-pair, i-chunk of 512). The two heads of a pair occupy
    SBUF partitions 0-63 / 64-127 of QT/KT, so their S^T matmuls issue as
    back-to-back row-tiled pairs (tile_position (0,0)/(64,0)) that execute
    CONCURRENTLY on the PE (HW-probed: 113 ns/MM vs 215 serial, 1.9x).
  - Part of the exp work moves off the ACT engine onto the DVE as a
    Schraudolph bit-trick: P_bf16bits = int16(rint(A*S + B)), one
    tensor_scalar (fp32 PSUM -> int16 SBUF, round-to-nearest verified on
    HW), bitcast to bf16 for the AV matmul. Host-simulated rel-err with
    this split: ~0.012 (gate 0.02).
  - Input DMAs are merged into few large 3D descriptors, spread across
    sync/vector/scalar/gpsimd queues, issued critical-first (wq+xq first).
  - PE warm-up dummies + early exp-table preload hide the HAM cold clock
    (4/8 = 1.2 GHz) and the 2.7us ACT table load during the input DMA.
  - Output blocks are staged 4-at-a-time in SBUF and written with one DMA
    per (head, 512-chunk): 16 output DMAs instead of 64.
"""

import numpy as np
import ml_dtypes

import concourse.bacc as bacc
import concourse.mybir as mybir
import concourse.tile as tile
from concourse.bass_utils import run_bass_kernel_spmd
from concourse.masks import make_identity

B, N, D, E = 4, 2048, 512, 512
H, CH = 8, 64
HPC = 4              # heads per core
EC = HPC * CH        # 256 E-columns per core
SCALE = 1.0 / 8.0    # 1/sqrt(CH)
NT = N // 128        # 16 j-tiles
DT = D // 128        # 4 d-tiles

SIGMA = 0.055
A_SCH = float(np.float32(128.0 * np.log2(np.e) * SCALE))
B_SCH = float(np.float32(128.0 * (127.0 - SIGMA)))
# Schraudolph j-tiles per pass (none in pass 0: it is projection-bound;
# j=15 keeps the pass-boundary st WAR off the last ACT)
SCH_BY_PASS = [()] + [(1, 5, 14, 15)] * 7

F32 = mybir.dt.float32
BF16 = mybir.dt.bfloat16
I16 = mybir.dt.int16
NP_BF16 = ml_dtypes.bfloat16

_cache = {}


def _build():
    nc = bacc.Bacc("TRN2", target_bir_lowering=False, debug=False)

    # x tensors host-interleaved: row p = [c-major][t-major][n'] so a
    # 512-column chunk is 4KB contiguous per row (DMA packets are
    # overhead-bound, so line size is the bandwidth lever)
    xq = nc.dram_tensor("xq", [128, DT * N], BF16, kind="ExternalInput")
    xk = nc.dram_tensor("xk", [128, DT * N], BF16, kind="ExternalInput")
    xv = nc.dram_tensor("xv", [128, DT * N], BF16, kind="ExternalInput")
    wq = nc.dram_tensor("wq", [128, DT * EC], BF16, kind="ExternalInput")
    wk = nc.dram_tensor("wk", [128, DT * EC], BF16, kind="ExternalInput")
    wv = nc.dram_tensor("wv", [128, DT * EC], BF16, kind="ExternalInput")
    bqc = nc.dram_tensor("bqc", [EC, 1], F32, kind="ExternalInput")
    bkc = nc.dram_tensor("bkc", [EC, 1], F32, kind="ExternalInput")
    bvr = nc.dram_tensor("bvr", [128, EC], F32, kind="ExternalInput")
    # output blocks land contiguous per (head, chunk); host reassembles
    out = nc.dram_tensor("out", [HPC * 4 * 128, 256], F32, kind="ExternalOutput")

    with tile.TileContext(nc) as tc:
        with (
            tc.tile_pool(name="singles", bufs=1) as singles,
            tc.tile_pool(name="qkv", bufs=1) as qkv,
            tc.tile_pool(name="fin", bufs=3) as fin_pool,
        ):
            # ---- SBUF staging ----
            dummy = singles.tile([128, 512], BF16, tag="dummy", name="dummy")
            # flat, chunk-major (c, t, n') so every chunk DMA is a 2D copy
            # with 4KB contiguous per partition (max DMA packet size)
            xq_sb = singles.tile([128, DT * N], BF16, tag="xq", name="xq")
            xk_sb = singles.tile([128, DT * N], BF16, tag="xk", name="xk")
            xv_sb = singles.tile([128, DT * N], BF16, tag="xv", name="xv")
            wq_sb = singles.tile([128, DT * EC], BF16, tag="wq", name="wq")
            wk_sb = singles.tile([128, DT * EC], BF16, tag="wk", name="wk")
            wv_sb = singles.tile([128, DT * EC], BF16, tag="wv", name="wv")
            bq_sb = [singles.tile([128, 1], F32, tag=f"bq{m}", name=f"bq{m}") for m in range(2)]
            bk_sb = [singles.tile([128, 1], F32, tag=f"bk{m}", name=f"bk{m}") for m in range(2)]
            bvr_sb = singles.tile([128, EC], F32, tag="bvr", name="bvr")
            ident = singles.tile([65, 65], F32, tag="ident", name="ident")

            # ---- engine warm-up (emitted first on their queues) ----
            nc.vector.memset(dummy, 0.0)
            gate_sb = singles.tile([1, 8], BF16, tag="gate", name="gate")

            # ---- input DMAs: merged descriptors, critical-first ----
            def xq_c(c):
                return (xq_sb[:, c * 2048:(c + 1) * 2048],
                        xq[:, c * 2048:(c + 1) * 2048])

            def xk_c(c):
                return (xk_sb[:, c * 2048:(c + 1) * 2048],
                        xk[:, c * 2048:(c + 1) * 2048])

            def xv_c(c):
                return (xv_sb[:, c * 2048:(c + 1) * 2048],
                        xv[:, c * 2048:(c + 1) * 2048])

            def cview(sb):  # [128, 4c*4t*512] -> [128, c, t, n']
                return sb.rearrange("p (c t n) -> p c t n", c=4, t=DT)

            def wview(sb):  # [128, 4t*EC] -> [128, t, e]
                return sb.rearrange("p (t e) -> p t e", t=DT)

            # wave 1 (ungated): QK projection critical path + V pass-0 needs.
            # Everything else is gated behind wave-1 arrival by gpsimd
            # *compute* ops (a DMA's sem-wait rides the descriptor, so a
            # gating DMA would not block later queue entries -- a tensor_copy
            # does).
            nc.sync.dma_start(wq_sb, wq[:, :])
            nc.sync.dma_start(*xq_c(0))
            nc.scalar.dma_start(bq_sb[0], bqc[0:128, :])
            nc.scalar.dma_start(wk_sb, wk[:, :])
            nc.scalar.dma_start(*xk_c(0))
            nc.scalar.dma_start(bk_sb[0], bkc[0:128, :])
            nc.scalar.dma_start(bq_sb[1], bqc[128:256, :])
            nc.scalar.dma_start(bk_sb[1], bkc[128:256, :])
            nc.gpsimd.dma_start(wv_sb, wv[:, :])
            nc.gpsimd.dma_start(*xv_c(0))
            nc.gpsimd.dma_start(bvr_sb, bvr[:, :])
            # ACT table preload, after scalar's DMA issues
            actwarm = singles.tile([1, 8], BF16, tag="actwarm", name="actwarm")
            nc.scalar.activation(
                actwarm, dummy[0:1, 0:8], mybir.ActivationFunctionType.Exp,
                scale=SCALE,
            )
            # wave 2: WAW-gated on xq c0 arrival (tiny pre-write into the
            # DMA destination forces the DMA to wait; emission-order gating
            # does not survive the scheduler)
            nc.vector.tensor_copy(xk_sb[0:1, 2048:2050], xq_sb[0:1, 0:2])
            nc.gpsimd.dma_start(*xk_c(1))

            # ---- working tiles ----
            qt_sb = [qkv.tile([128, N], BF16, tag=f"qt{m}", name=f"qt{m}") for m in range(2)]
            kt_sb = [qkv.tile([128, N], BF16, tag=f"kt{m}", name=f"kt{m}") for m in range(2)]
            v_sb = [qkv.tile([128, HPC * 65], BF16, tag=f"v{t}", name=f"v{t}") for t in range(NT)]
            for t in range(NT):
                ones_view = v_sb[t].rearrange("p (h c) -> p h c", c=65)[:, :, 64:65]
                nc.vector.memset(ones_view, 1.0)
            ots_sb = [qkv.tile([65, N], F32, tag=f"ots{h}", name=f"ots{h}") for h in range(HPC)]
            make_identity(nc, ident)

            with (
                tc.tile_pool(name="proj_ps", bufs=2, space="PSUM") as proj_ps,
                tc.tile_pool(name="st_ps", bufs=2, space="PSUM") as st_ps,
                tc.tile_pool(name="ot_ps", bufs=1, space="PSUM") as ot_ps,
                tc.tile_pool(name="pt_sb", bufs=4) as pt_pool,
                tc.tile_pool(name="pti_sb", bufs=2) as pti_pool,
            ):
                # PE warm-up: ~14 dummy matmuls flip HAM to 8/8 during DMA
                for i in range(9):
                    ps = proj_ps.tile([128, 512], F32, tag="proj", name="warm")
                    nc.tensor.matmul(ps, lhsT=dummy[:, 0:128], rhs=dummy,
                                     start=True, stop=True)

                # -- emitters --
                def emit_qk_group(dst, w_s, x_s, b_s, hp, nch):
                    ps = proj_ps.tile([128, 512], F32, tag="proj", name="qkp")
                    for t in range(DT):
                        nc.tensor.matmul(
                            ps,
                            lhsT=wview(w_s)[:, t, hp * 128:(hp + 1) * 128],
                            rhs=cview(x_s)[:, nch, t, :],
                            start=(t == 0),
                            stop=(t == DT - 1),
                        )
                    nc.vector.tensor_scalar_add(
                        dst[hp][:, nch * 512:(nch + 1) * 512], ps, b_s[hp]
                    )

                def emit_v_group(t):
                    ps = proj_ps.tile([128, EC], F32, tag="proj", name="vp")
                    for d in range(DT):
                        nc.tensor.matmul(
                            ps,
                            lhsT=cview(xv_sb)[:, t // 4, d,
                                              (t % 4) * 128:(t % 4 + 1) * 128],
                            rhs=wview(wv_sb)[:, d, :],
                            start=(d == 0),
                            stop=(d == DT - 1),
                        )
                    v_view = v_sb[t].rearrange("p (h c) -> p h c", c=65)[:, :, 0:64]
                    nc.vector.tensor_add(
                        v_view,
                        ps.rearrange("p (h c) -> p h c", c=64),
                        bvr_sb.rearrange("p (h c) -> p h c", c=64),
                    )

                def emit_filler(f):
                    if f[0] == "v":
                        emit_v_group(f[1])
                    elif f[0] == "q":
                        emit_qk_group(qt_sb, wq_sb, xq_sb, bq_sb, f[1], f[2])
                    else:
                        emit_qk_group(kt_sb, wk_sb, xk_sb, bk_sb, f[1], f[2])

                def emit_s_pair(hp, icol, j):
                    st = st_ps.tile([128, 1024], F32, tag="st", name="st")
                    for half in range(2):
                        ho = half * 64
                        nc.tensor.matmul(
                            st[:, half * 512:(half + 1) * 512],
                            lhsT=kt_sb[hp][ho:ho + 64, j * 128:(j + 1) * 128],
                            rhs=qt_sb[hp][ho:ho + 64, icol:icol + 512],
                            start=True,
                            stop=True,
                        )
                    return st

                ob_state = {}

                def emit_out_tr(hd, c, t):
                    # one transpose step of head hd's OT chunk c
                    if t == 0:
                        ob_state[(hd, c)] = proj_ps.tile(
                            [128, 512], F32, tag="proj", name="tr"
                        )
                    tr = ob_state[(hd, c)]
                    nc.tensor.transpose(
                        tr[:, t * 65:(t + 1) * 65],
                        ots_sb[hd][:, (c * 4 + t) * 128:(c * 4 + t + 1) * 128],
                        ident,
                    )

                def emit_out_fin(hd, c):
                    # batched recip, 4 muls, one DMA for the finished group
                    tr = ob_state.pop((hd, c))
                    tr3 = tr[:, 0:260].rearrange("p (t c) -> p t c", c=65)
                    rec = fin_pool.tile([128, 4], F32, tag="rec", name="rec")
                    nc.vector.reciprocal(rec, tr3[:, :, 64])
                    otile = fin_pool.tile([128, 256], F32, tag="otile", name="otile")
                    for t in range(4):
                        nc.vector.tensor_scalar_mul(
                            otile[:, t * 64:(t + 1) * 64],
                            tr3[:, t, 0:64],
                            rec[:, t:t + 1],
                        )
                    blk = (hd * 4 + c) * 128
                    eng = nc.sync if (hd + c) % 2 == 0 else nc.gpsimd
                    eng.dma_start(out[blk:blk + 128, :], otile)

                # filler schedule: (pass, iter) -> list of jobs
                fillers = {
                    (0, 0): [("v", 0)], (0, 1): [("k", 0, 1), ("v", 1)],
                    (0, 2): [("v", 2), ("v", 3)], (0, 3): [("v", 4)],
                    (0, 4): [("v", 5), ("v", 6)], (0, 5): [("k", 0, 2)],
                    (0, 6): [("v", 7), ("v", 8)], (0, 7): [("v", 9)],
                    (0, 8): [("v", 10), ("v", 11)], (0, 9): [("k", 0, 3)],
                    (0, 10): [("v", 12), ("v", 13)],
                    (0, 11): [("q", 0, 1)],
                    (0, 12): [("v", 14)], (0, 13): [("v", 15)],
                    (1, 7): [("q", 0, 2)], (1, 13): [("k", 1, 0)],
                    (1, 14): [("k", 1, 1)],
                    (2, 7): [("q", 0, 3)], (2, 13): [("k", 1, 2)],
                    (2, 14): [("k", 1, 3)],
                    (3, 7): [("q", 1, 0)],
                    (4, 7): [("q", 1, 1)],
                    (5, 7): [("q", 1, 2)],
                    (6, 7): [("q", 1, 3)],
                }
                # out-block steps: (pass, iter) -> ("tr", head, chunk, t)
                # or ("fin", head, chunk); transposes spread one per iter
                obsteps = {}
                for p in range(1, 8):
                    hp_prev, c_prev = (p - 1) // 4, (p - 1) % 4
                    for t in range(4):
                        obsteps[(p, 2 + t)] = ("tr", 2 * hp_prev, c_prev, t)
                        obsteps[(p, 8 + t)] = ("tr", 2 * hp_prev + 1, c_prev, t)
                    obsteps[(p, 6)] = ("fin", 2 * hp_prev, c_prev)
                    obsteps[(p, 12)] = ("fin", 2 * hp_prev + 1, c_prev)

                # -- prologue for pass 0 --
                emit_qk_group(qt_sb, wq_sb, xq_sb, bq_sb, 0, 0)
                emit_qk_group(kt_sb, wk_sb, xk_sb, bk_sb, 0, 0)

                # wave 3 inputs: WAW-gated on the first QT chunk
                qg = qt_sb[0][0:1, 0:2]
                for sb, cc in ((xk_sb, 2), (xv_sb, 1), (xk_sb, 3),
                               (xq_sb, 1), (xv_sb, 2), (xq_sb, 2),
                               (xv_sb, 3), (xq_sb, 3)):
                    nc.vector.tensor_copy(
                        sb[0:1, cc * 2048:cc * 2048 + 2], qg)
                nc.gpsimd.dma_start(*xk_c(2))
                nc.gpsimd.dma_start(*xv_c(1))
                nc.gpsimd.dma_start(*xk_c(3))
                nc.gpsimd.dma_start(*xq_c(1))
                nc.gpsimd.dma_start(*xv_c(2))
                nc.gpsimd.dma_start(*xq_c(2))
                nc.gpsimd.dma_start(*xv_c(3))
                nc.gpsimd.dma_start(*xq_c(3))

                prologue = [None, None]
                for p in range(8):
                    hp, c = p // 4, p % 4
                    icol = c * 512
                    ha, hb = 2 * hp, 2 * hp + 1
                    sch = SCH_BY_PASS[p]
                    ot = ot_ps.tile([65, 1024], F32, tag="ot", name="ot")
                    sts = [None] * NT
                    pts = [None] * NT

                    if p == 0:
                        sts[0] = emit_s_pair(hp, icol, 0)
                        sts[1] = emit_s_pair(hp, icol, 1)
                    else:
                        sts[0], sts[1] = prologue

                    def emit_av_pair(j):
                        for half, hd in ((0, ha), (1, hb)):
                            nc.tensor.matmul(
                                ot[:, half * 512:(half + 1) * 512],
                                lhsT=v_sb[j][:, hd * 65:(hd + 1) * 65],
                                rhs=pts[j][:, half * 512:(half + 1) * 512],
                                start=(j == 0),
                                stop=(j == NT - 1),
                            )

                    def emit_sch(j):
                        # Schraudolph exp on DVE, one iter ahead of its slot
                        # so the st-buffer WAR never stalls the S pipeline
                        pti = pti_pool.tile([128, 1024], I16, tag="pti", name="pti")
                        nc.vector.tensor_scalar(
                            pti, sts[j], A_SCH, B_SCH,
                            mybir.AluOpType.mult, mybir.AluOpType.add,
                        )
                        pts[j] = pti.bitcast(BF16)

                    if 0 in sch:
                        emit_sch(0)
                    for j in range(NT):
                        if j not in sch:
                            pt = pt_pool.tile([128, 1024], BF16, tag="pt", name="pt")
                            nc.scalar.activation(
                                pt, sts[j], mybir.ActivationFunctionType.Exp,
                                scale=SCALE,
                            )
                            pts[j] = pt
                        if j + 1 in sch:
                            emit_sch(j + 1)
                        if j % 2 == 1:
                            if j >= 2:
                                emit_av_pair(j - 2)
                            emit_av_pair(j - 1)
                        if j % 2 == 0:
                            for jj in (j + 2, j + 3):
                                if jj < NT:
                                    sts[jj] = emit_s_pair(hp, icol, jj)
                                elif p + 1 < 8:
                                    nhp, nc_ = (p + 1) // 4, (p + 1) % 4
                                    prologue[jj - NT] = emit_s_pair(
                                        nhp, nc_ * 512, jj - NT)
                        for f in fillers.get((p, j), ()):
                            emit_filler(f)
                        step = obsteps.get((p, j))
                        if step is not None:
                            if step[0] == "tr":
                                emit_out_tr(step[1], step[2], step[3])
                            else:
                                emit_out_fin(step[1], step[2])

                    emit_av_pair(NT - 1)
                    nc.vector.tensor_copy(ots_sb[ha][:, icol:icol + 512], ot[:, 0:512])
                    nc.vector.tensor_copy(ots_sb[hb][:, icol:icol + 512], ot[:, 512:1024])

                # tail: last pair's final chunk
                for t in range(4):
                    emit_out_tr(2, 3, t)
                emit_out_fin(2, 3)
                for t in range(4):
                    emit_out_tr(3, 3, t)
                emit_out_fin(3, 3)

    nc.compile()
    return nc


def _get_nc():
    if "nc" not in _cache:
        _cache["nc"] = _build()
    return _cache["nc"]


def _ilv_x(xT):
    # [D, N] -> [128, 4c * 4t * 512n'] with row p = [c][t][n'] interleave
    return np.ascontiguousarray(
        xT.reshape(DT, 128, 4, 512).transpose(1, 2, 0, 3).reshape(128, DT * N)
    ).astype(NP_BF16)


def _ilv_w(w):
    # [D, EC] -> [128, 4t * EC]
    return np.ascontiguousarray(
        w.reshape(DT, 128, EC).transpose(1, 0, 2).reshape(128, DT * EC)
    ).astype(NP_BF16)


def _shard_inputs(q, k, v, Wq, Wk, Wv, bq, bk, bv):
    in_maps = []
    for c in range(8):
        b, g = c // 2, c % 2
        sl = slice(g * EC, (g + 1) * EC)
        in_maps.append({
            "xq": _ilv_x(np.asarray(q)[b].T),
            "xk": _ilv_x(np.asarray(k)[b].T),
            "xv": _ilv_x(np.asarray(v)[b].T),
            "wq": _ilv_w(np.asarray(Wq)[:, sl]),
            "wk": _ilv_w(np.asarray(Wk)[:, sl]),
            "wv": _ilv_w(np.asarray(Wv)[:, sl]),
            "bqc": np.asarray(bq)[sl].reshape(EC, 1).astype(np.float32),
            "bkc": np.asarray(bk)[sl].reshape(EC, 1).astype(np.float32),
            "bvr": np.ascontiguousarray(
                np.broadcast_to(np.asarray(bv)[sl], (128, EC))
            ).astype(np.float32),
        })
    return in_maps


def kernel(q, k, v, Wq, Wk, Wv, bq, bk, bv, _trace=False):
    nc = _get_nc()
    in_maps = _shard_inputs(q, k, v, Wq, Wk, Wv, bq, bk, bv)
    res = run_bass_kernel_spmd(
        nc, in_maps, core_ids=list(range(8)), trace=_trace
    )
    out = np.empty((B, N, E), np.float32)
    for c in range(8):
        b, g = c // 2, c % 2
        o2 = res.results[c]["out"].reshape(HPC, 4, 128, 4, 64)
        out[b, :, g * EC:(g + 1) * EC] = (
            o2.transpose(1, 3, 2, 0, 4).reshape(N, EC)
        )
    if _trace:
        _cache["last_exec_time_ns"] = res.exec_time_ns
    return out


# revision 26
# speedup vs baseline: 1.0024x; 1.0024x over previous
"""Multi-head attention TRN2 Bass kernel (v2).

Problem: B=4, N=2048, D=E=512, 8 heads (ch=64).
out = softmax((x_q Wq + bq)(x_k Wk + bk)^T / 8) (x_v Wv + bv), per head.

Sharding (8 cores): core c handles batch b = c//2 and head-group g = c%2
(4 heads = 256 E-columns). Each core is fully independent (no collectives).

v2 changes over the original ACT-paced design:
  - Pass = (head-pair, i-chunk of 512). The two heads of a pair occupy
    SBUF partitions 0-63 / 64-127 of QT/KT, so their S^T matmuls issue as
    back-to-back row-tiled pairs (tile_position (0,0)/(64,0)) that execute
    CONCURRENTLY on the PE (HW-probed: 113 ns/MM vs 215 serial, 1.9x).
  - Part of the exp work moves off the ACT engine onto the DVE as a
    Schraudolph bit-trick: P_bf16bits = int16(rint(A*S + B)), one
    tensor_scalar (fp32 PSUM -> int16 SBUF, round-to-nearest verified on
    HW), bitcast to bf16 for the AV matmul. Host-simulated rel-err with
    this split: ~0.012 (gate 0.02).
  - Input DMAs are merged into few large 3D descriptors, spread across
    sync/vector/scalar/gpsimd queues, issued critical-first (wq+xq first).
  - PE warm-up dummies + early exp-table preload hide the HAM cold clock
    (4/8 = 1.2 GHz) and the 2.7us ACT table load during the input DMA.
  - Output blocks are staged 4-at-a-time in SBUF and written with one DMA
    per (head, 512-chunk): 16 output DMAs instead of 64.
"""

import numpy as np
import ml_dtypes

import concourse.bacc as bacc
import concourse.mybir as mybir
import concourse.tile as tile
from concourse.bass_utils import run_bass_kernel_spmd
from concourse.masks import make_identity

B, N, D, E = 4, 2048, 512, 512
H, CH = 8, 64
HPC = 4              # heads per core
EC = HPC * CH        # 256 E-columns per core
SCALE = 1.0 / 8.0    # 1/sqrt(CH)
NT = N // 128        # 16 j-tiles
DT = D // 128        # 4 d-tiles

SIGMA = 0.055
A_SCH = float(np.float32(128.0 * np.log2(np.e) * SCALE))
B_SCH = float(np.float32(128.0 * (127.0 - SIGMA)))
# Schraudolph j-tiles per pass (none in pass 0: it is projection-bound;
# j=15 keeps the pass-boundary st WAR off the last ACT)
SCH_BY_PASS = [()] + [(1, 5, 14, 15)] * 7

F32 = mybir.dt.float32
BF16 = mybir.dt.bfloat16
I16 = mybir.dt.int16
NP_BF16 = ml_dtypes.bfloat16

_cache = {}


def _build():
    nc = bacc.Bacc("TRN2", target_bir_lowering=False, debug=False)

    # x tensors host-interleaved: row p = [c-major][t-major][n'] so a
    # 512-column chunk is 4KB contiguous per row (DMA packets are
    # overhead-bound, so line size is the bandwidth lever)
    xq = nc.dram_tensor("xq", [128, DT * N], BF16, kind="ExternalInput")
    xk = nc.dram_tensor("xk", [128, DT * N], BF16, kind="ExternalInput")
    xv = nc.dram_tensor("xv", [128, DT * N], BF16, kind="ExternalInput")
    wq = nc.dram_tensor("wq", [128, DT * EC], BF16, kind="ExternalInput")
    wk = nc.dram_tensor("wk", [128, DT * EC], BF16, kind="ExternalInput")
    wv = nc.dram_tensor("wv", [128, DT * EC], BF16, kind="ExternalInput")
    # all four bias vectors as one [128, 4] tensor (cols: bq m0, bq m1,
    # bk m0, bk m1) -- a [128,1] DMA degenerates to 4-byte packets
    bmat = nc.dram_tensor("bmat", [128, 4], F32, kind="ExternalInput")
    bvr = nc.dram_tensor("bvr", [128, EC], F32, kind="ExternalInput")
    # output blocks land contiguous per (head, chunk); host reassembles
    out = nc.dram_tensor("out", [HPC * 4 * 128, 256], F32, kind="ExternalOutput")

    with tile.TileContext(nc) as tc:
        with (
            tc.tile_pool(name="singles", bufs=1) as singles,
            tc.tile_pool(name="qkv", bufs=1) as qkv,
            tc.tile_pool(name="fin", bufs=3) as fin_pool,
        ):
            # ---- SBUF staging ----
            dummy = singles.tile([128, 512], BF16, tag="dummy", name="dummy")
            # flat, chunk-major (c, t, n') so every chunk DMA is a 2D copy
            # with 4KB contiguous per partition (max DMA packet size)
            xq_sb = singles.tile([128, DT * N], BF16, tag="xq", name="xq")
            xk_sb = singles.tile([128, DT * N], BF16, tag="xk", name="xk")
            xv_sb = singles.tile([128, DT * N], BF16, tag="xv", name="xv")
            wq_sb = singles.tile([128, DT * EC], BF16, tag="wq", name="wq")
            wk_sb = singles.tile([128, DT * EC], BF16, tag="wk", name="wk")
            wv_sb = singles.tile([128, DT * EC], BF16, tag="wv", name="wv")
            bm_sb = singles.tile([128, 4], F32, tag="bm", name="bm")
            bq_sb = [bm_sb[:, m:m + 1] for m in range(2)]
            bk_sb = [bm_sb[:, 2 + m:3 + m] for m in range(2)]
            bvr_sb = singles.tile([128, EC], F32, tag="bvr", name="bvr")
            ident = singles.tile([65, 65], F32, tag="ident", name="ident")

            # ---- engine warm-up (emitted first on their queues) ----
            nc.vector.memset(dummy, 0.0)
            gate_sb = singles.tile([1, 8], BF16, tag="gate", name="gate")

            # ---- input DMAs: merged descriptors, critical-first ----
            def xq_c(c):
                return (xq_sb[:, c * 2048:(c + 1) * 2048],
                        xq[:, c * 2048:(c + 1) * 2048])

            def xk_c(c):
                return (xk_sb[:, c * 2048:(c + 1) * 2048],
                        xk[:, c * 2048:(c + 1) * 2048])

            def xv_c(c):
                return (xv_sb[:, c * 2048:(c + 1) * 2048],
                        xv[:, c * 2048:(c + 1) * 2048])

            def cview(sb):  # [128, 4c*4t*512] -> [128, c, t, n']
                return sb.rearrange("p (c t n) -> p c t n", c=4, t=DT)

            def wview(sb):  # [128, 4t*EC] -> [128, t, e]
                return sb.rearrange("p (t e) -> p t e", t=DT)

            # wave 1 (ungated): QK projection critical path + V pass-0 needs.
            # Everything else is gated behind wave-1 arrival by gpsimd
            # *compute* ops (a DMA's sem-wait rides the descriptor, so a
            # gating DMA would not block later queue entries -- a tensor_copy
            # does).
            nc.sync.dma_start(wq_sb, wq[:, :])
            nc.sync.dma_start(*xq_c(0))
            nc.scalar.dma_start(bm_sb, bmat[:, :])
            nc.scalar.dma_start(wk_sb, wk[:, :])
            nc.scalar.dma_start(*xk_c(0))
            nc.gpsimd.dma_start(wv_sb, wv[:, :])
            nc.gpsimd.dma_start(*xv_c(0))
            nc.gpsimd.dma_start(bvr_sb, bvr[:, :])
            # ACT table preload, after scalar's DMA issues
            actwarm = singles.tile([1, 8], BF16, tag="actwarm", name="actwarm")
            nc.scalar.activation(
                actwarm, dummy[0:1, 0:8], mybir.ActivationFunctionType.Exp,
                scale=SCALE,
            )
            # wave 2: WAW-gated on xq c0 arrival (tiny pre-write into the
            # DMA destination forces the DMA to wait; emission-order gating
            # does not survive the scheduler)
            nc.vector.tensor_copy(xk_sb[0:1, 2048:2050], xq_sb[0:1, 0:2])
            nc.gpsimd.dma_start(*xk_c(1))

            # ---- working tiles ----
            qt_sb = [qkv.tile([128, N], BF16, tag=f"qt{m}", name=f"qt{m}") for m in range(2)]
            kt_sb = [qkv.tile([128, N], BF16, tag=f"kt{m}", name=f"kt{m}") for m in range(2)]
            v_sb = [qkv.tile([128, HPC * 65], BF16, tag=f"v{t}", name=f"v{t}") for t in range(NT)]
            for t in range(NT):
                ones_view = v_sb[t].rearrange("p (h c) -> p h c", c=65)[:, :, 64:65]
                nc.vector.memset(ones_view, 1.0)
            ots_sb = [qkv.tile([65, N], F32, tag=f"ots{h}", name=f"ots{h}") for h in range(HPC)]
            make_identity(nc, ident)

            with (
                tc.tile_pool(name="proj_ps", bufs=2, space="PSUM") as proj_ps,
                tc.tile_pool(name="st_ps", bufs=2, space="PSUM") as st_ps,
                tc.tile_pool(name="ot_ps", bufs=1, space="PSUM") as ot_ps,
                tc.tile_pool(name="pt_sb", bufs=4) as pt_pool,
                tc.tile_pool(name="pti_sb", bufs=2) as pti_pool,
            ):
                # PE warm-up: ~14 dummy matmuls flip HAM to 8/8 during DMA
                for i in range(9):
                    ps = proj_ps.tile([128, 512], F32, tag="proj", name="warm")
                    nc.tensor.matmul(ps, lhsT=dummy[:, 0:128], rhs=dummy,
                                     start=True, stop=True)

                # -- emitters --
                def emit_qk_group(dst, w_s, x_s, b_s, hp, nch):
                    ps = proj_ps.tile([128, 512], F32, tag="proj", name="qkp")
                    for t in range(DT):
                        nc.tensor.matmul(
                            ps,
                            lhsT=wview(w_s)[:, t, hp * 128:(hp + 1) * 128],
                            rhs=cview(x_s)[:, nch, t, :],
                            start=(t == 0),
                            stop=(t == DT - 1),
                        )
                    nc.vector.tensor_scalar_add(
                        dst[hp][:, nch * 512:(nch + 1) * 512], ps, b_s[hp]
                    )

                def emit_v_group(t):
                    ps = proj_ps.tile([128, EC], F32, tag="proj", name="vp")
                    for d in range(DT):
                        nc.tensor.matmul(
                            ps,
                            lhsT=cview(xv_sb)[:, t // 4, d,
                                              (t % 4) * 128:(t % 4 + 1) * 128],
                            rhs=wview(wv_sb)[:, d, :],
                            start=(d == 0),
                            stop=(d == DT - 1),
                        )
                    v_view = v_sb[t].rearrange("p (h c) -> p h c", c=65)[:, :, 0:64]
                    nc.vector.tensor_add(
                        v_view,
                        ps.rearrange("p (h c) -> p h c", c=64),
                        bvr_sb.rearrange("p (h c) -> p h c", c=64),
                    )

                def emit_filler(f):
                    if f[0] == "v":
                        emit_v_group(f[1])
                    elif f[0] == "q":
                        emit_qk_group(qt_sb, wq_sb, xq_sb, bq_sb, f[1], f[2])
                    else:
                        emit_qk_group(kt_sb, wk_sb, xk_sb, bk_sb, f[1], f[2])

                def emit_s_pair(hp, icol, j):
                    st = st_ps.tile([128, 1024], F32, tag="st", name="st")
                    for half in range(2):
                        ho = half * 64
                        nc.tensor.matmul(
                            st[:, half * 512:(half + 1) * 512],
                            lhsT=kt_sb[hp][ho:ho + 64, j * 128:(j + 1) * 128],
                            rhs=qt_sb[hp][ho:ho + 64, icol:icol + 512],
                            start=True,
                            stop=True,
                        )
                    return st

                ob_state = {}

                def emit_out_tr(hd, c, t):
                    # one transpose step of head hd's OT chunk c
                    if t == 0:
                        ob_state[(hd, c)] = proj_ps.tile(
                            [128, 512], F32, tag="proj", name="tr"
                        )
                    tr = ob_state[(hd, c)]
                    nc.tensor.transpose(
                        tr[:, t * 65:(t + 1) * 65],
                        ots_sb[hd][:, (c * 4 + t) * 128:(c * 4 + t + 1) * 128],
                        ident,
                    )

                def emit_out_fin(hd, c):
                    # batched recip, 4 muls, one DMA for the finished group
                    tr = ob_state.pop((hd, c))
                    tr3 = tr[:, 0:260].rearrange("p (t c) -> p t c", c=65)
                    rec = fin_pool.tile([128, 4], F32, tag="rec", name="rec")
                    nc.vector.reciprocal(rec, tr3[:, :, 64])
                    otile = fin_pool.tile([128, 256], F32, tag="otile", name="otile")
                    for t in range(4):
                        nc.vector.tensor_scalar_mul(
                            otile[:, t * 64:(t + 1) * 64],
                            tr3[:, t, 0:64],
                            rec[:, t:t + 1],
                        )
                    blk = (hd * 4 + c) * 128
                    eng = nc.sync if (hd + c) % 2 == 0 else nc.gpsimd
                    eng.dma_start(out[blk:blk + 128, :], otile)

                # filler schedule: (pass, iter) -> list of jobs
                fillers = {
                    (0, 0): [("v", 0)], (0, 1): [("k", 0, 1), ("v", 1)],
                    (0, 2): [("v", 2), ("v", 3)], (0, 3): [("v", 4)],
                    (0, 4): [("v", 5), ("v", 6)], (0, 5): [("k", 0, 2)],
                    (0, 6): [("v", 7), ("v", 8)], (0, 7): [("v", 9)],
                    (0, 8): [("v", 10), ("v", 11)], (0, 9): [("k", 0, 3)],
                    (0, 10): [("v", 12), ("v", 13)],
                    (0, 11): [("q", 0, 1)],
                    (0, 12): [("v", 14)], (0, 13): [("v", 15)],
                    (1, 7): [("q", 0, 2)], (1, 13): [("k", 1, 0)],
                    (1, 14): [("k", 1, 1)],
                    (2, 7): [("q", 0, 3)], (2, 13): [("k", 1, 2)],
                    (2, 14): [("k", 1, 3)],
                    (3, 7): [("q", 1, 0)],
                    (4, 7): [("q", 1, 1)],
                    (5, 7): [("q", 1, 2)],
                    (6, 7): [("q", 1, 3)],
                }
                # out-block steps: (pass, iter) -> ("tr", head, chunk, t)
                # or ("fin", head, chunk); transposes spread one per iter
                obsteps = {}
                for p in range(1, 8):
                    hp_prev, c_prev = (p - 1) // 4, (p - 1) % 4
                    for t in range(4):
                        obsteps[(p, 2 + t)] = ("tr", 2 * hp_prev, c_prev, t)
                        obsteps[(p, 8 + t)] = ("tr", 2 * hp_prev + 1, c_prev, t)
                    obsteps[(p, 6)] = ("fin", 2 * hp_prev, c_prev)
                    obsteps[(p, 12)] = ("fin", 2 * hp_prev + 1, c_prev)

                # -- prologue for pass 0 --
                emit_qk_group(qt_sb, wq_sb, xq_sb, bq_sb, 0, 0)
                emit_qk_group(kt_sb, wk_sb, xk_sb, bk_sb, 0, 0)

                # wave 3 inputs: WAW-gated on the first QT chunk
                qg = qt_sb[0][0:1, 0:2]
                for sb, cc in ((xk_sb, 2), (xv_sb, 1), (xk_sb, 3),
                               (xq_sb, 1), (xv_sb, 2), (xq_sb, 2),
                               (xv_sb, 3), (xq_sb, 3)):
                    nc.vector.tensor_copy(
                        sb[0:1, cc * 2048:cc * 2048 + 2], qg)
                nc.gpsimd.dma_start(*xk_c(2))
                nc.gpsimd.dma_start(*xv_c(1))
                nc.gpsimd.dma_start(*xk_c(3))
                nc.gpsimd.dma_start(*xq_c(1))
                nc.gpsimd.dma_start(*xv_c(2))
                nc.gpsimd.dma_start(*xq_c(2))
                nc.gpsimd.dma_start(*xv_c(3))
                nc.gpsimd.dma_start(*xq_c(3))

                prologue = [None, None]
                for p in range(8):
                    hp, c = p // 4, p % 4
                    icol = c * 512
                    ha, hb = 2 * hp, 2 * hp + 1
                    sch = SCH_BY_PASS[p]
                    ot = ot_ps.tile([65, 1024], F32, tag="ot", name="ot")
                    sts = [None] * NT
                    pts = [None] * NT

                    if p == 0:
                        sts[0] = emit_s_pair(hp, icol, 0)
                        sts[1] = emit_s_pair(hp, icol, 1)
                    else:
                        sts[0], sts[1] = prologue

                    def emit_av_pair(j):
                        for half, hd in ((0, ha), (1, hb)):
                            nc.tensor.matmul(
                                ot[:, half * 512:(half + 1) * 512],
                                lhsT=v_sb[j][:, hd * 65:(hd + 1) * 65],
                                rhs=pts[j][:, half * 512:(half + 1) * 512],
                                start=(j == 0),
                                stop=(j == NT - 1),
                            )

                    def emit_sch(j):
                        # Schraudolph exp on DVE, one iter ahead of its slot
                        # so the st-buffer WAR never stalls the S pipeline
                        pti = pti_pool.tile([128, 1024], I16, tag="pti", name="pti")
                        nc.vector.tensor_scalar(
                            pti, sts[j], A_SCH, B_SCH,
                            mybir.AluOpType.mult, mybir.AluOpType.add,
                        )
                        pts[j] = pti.bitcast(BF16)

                    if 0 in sch:
                        emit_sch(0)
                    for j in range(NT):
                        if j not in sch:
                            pt = pt_pool.tile([128, 1024], BF16, tag="pt", name="pt")
                            nc.scalar.activation(
                                pt, sts[j], mybir.ActivationFunctionType.Exp,
                                scale=SCALE,
                            )
                            pts[j] = pt
                        if j + 1 in sch:
                            emit_sch(j + 1)
                        if j % 2 == 1:
                            if j >= 2:
                                emit_av_pair(j - 2)
                            emit_av_pair(j - 1)
                        if j % 2 == 0:
                            for jj in (j + 2, j + 3):
                                if jj < NT:
                                    sts[jj] = emit_s_pair(hp, icol, jj)
                                elif p + 1 < 8:
                                    nhp, nc_ = (p + 1) // 4, (p + 1) % 4
                                    prologue[jj - NT] = emit_s_pair(
                                        nhp, nc_ * 512, jj - NT)
                        for f in fillers.get((p, j), ()):
                            emit_filler(f)
                        step = obsteps.get((p, j))
                        if step is not None:
                            if step[0] == "tr":
                                emit_out_tr(step[1], step[2], step[3])
                            else:
                                emit_out_fin(step[1], step[2])

                    emit_av_pair(NT - 1)
                    nc.vector.tensor_copy(ots_sb[ha][:, icol:icol + 512], ot[:, 0:512])
                    nc.vector.tensor_copy(ots_sb[hb][:, icol:icol + 512], ot[:, 512:1024])

                # tail: last pair's final chunk
                for t in range(4):
                    emit_out_tr(2, 3, t)
                emit_out_fin(2, 3)
                for t in range(4):
                    emit_out_tr(3, 3, t)
                emit_out_fin(3, 3)

    nc.compile()
    return nc


def _get_nc():
    if "nc" not in _cache:
        _cache["nc"] = _build()
    return _cache["nc"]


def _ilv_x(xT):
    # [D, N] -> [128, 4c * 4t * 512n'] with row p = [c][t][n'] interleave
    return np.ascontiguousarray(
        xT.reshape(DT, 128, 4, 512).transpose(1, 2, 0, 3).reshape(128, DT * N)
    ).astype(NP_BF16)


def _ilv_w(w):
    # [D, EC] -> [128, 4t * EC]
    return np.ascontiguousarray(
        w.reshape(DT, 128, EC).transpose(1, 0, 2).reshape(128, DT * EC)
    ).astype(NP_BF16)


def _shard_inputs(q, k, v, Wq, Wk, Wv, bq, bk, bv):
    in_maps = []
    for c in range(8):
        b, g = c // 2, c % 2
        sl = slice(g * EC, (g + 1) * EC)
        in_maps.append({
            "xq": _ilv_x(np.asarray(q)[b].T),
            "xk": _ilv_x(np.asarray(k)[b].T),
            "xv": _ilv_x(np.asarray(v)[b].T),
            "wq": _ilv_w(np.asarray(Wq)[:, sl]),
            "wk": _ilv_w(np.asarray(Wk)[:, sl]),
            "wv": _ilv_w(np.asarray(Wv)[:, sl]),
            "bmat": np.stack([
                np.asarray(bq)[sl][0:128], np.asarray(bq)[sl][128:256],
                np.asarray(bk)[sl][0:128], np.asarray(bk)[sl][128:256],
            ], axis=1).astype(np.float32),
            "bvr": np.ascontiguousarray(
                np.broadcast_to(np.asarray(bv)[sl], (128, EC))
            ).astype(np.float32),
        })
    return in_maps


def kernel(q, k, v, Wq, Wk, Wv, bq, bk, bv, _trace=False):
    nc = _get_nc()
    in_maps = _shard_inputs(q, k, v, Wq, Wk, Wv, bq, bk, bv)
    res = run_bass_kernel_spmd(
        nc, in_maps, core_ids=list(range(8)), trace=_trace
    )
    out = np.empty((B, N, E), np.float32)
    for c in range(8):
        b, g = c // 2, c % 2
        o2 = res.results[c]["out"].reshape(HPC, 4, 128, 4, 64)
        out[b, :, g * EC:(g + 1) * EC] = (
            o2.transpose(1, 3, 2, 0, 4).reshape(N, EC)
        )
    if _trace:
        _cache["last_exec_time_ns"] = res.exec_time_ns
    return out


# revision 29
# speedup vs baseline: 1.0095x; 1.0072x over previous
"""Multi-head attention TRN2 Bass kernel (v2).

Problem: B=4, N=2048, D=E=512, 8 heads (ch=64).
out = softmax((x_q Wq + bq)(x_k Wk + bk)^T / 8) (x_v Wv + bv), per head.

Sharding (8 cores): core c handles batch b = c//2 and head-group g = c%2
(4 heads = 256 E-columns). Each core is fully independent (no collectives).

v2 changes over the original ACT-paced design:
  - Pass = (head-pair, i-chunk of 512). The two heads of a pair occupy
    SBUF partitions 0-63 / 64-127 of QT/KT, so their S^T matmuls issue as
    back-to-back row-tiled pairs (tile_position (0,0)/(64,0)) that execute
    CONCURRENTLY on the PE (HW-probed: 113 ns/MM vs 215 serial, 1.9x).
  - Part of the exp work moves off the ACT engine onto the DVE as a
    Schraudolph bit-trick: P_bf16bits = int16(rint(A*S + B)), one
    tensor_scalar (fp32 PSUM -> int16 SBUF, round-to-nearest verified on
    HW), bitcast to bf16 for the AV matmul. Host-simulated rel-err with
    this split: ~0.012 (gate 0.02).
  - Input DMAs are merged into few large 3D descriptors, spread across
    sync/vector/scalar/gpsimd queues, issued critical-first (wq+xq first).
  - PE warm-up dummies + early exp-table preload hide the HAM cold clock
    (4/8 = 1.2 GHz) and the 2.7us ACT table load during the input DMA.
  - Output blocks are staged 4-at-a-time in SBUF and written with one DMA
    per (head, 512-chunk): 16 output DMAs instead of 64.
"""

import numpy as np
import ml_dtypes

import concourse.bacc as bacc
import concourse.mybir as mybir
import concourse.tile as tile
from concourse.bass_utils import run_bass_kernel_spmd
from concourse.masks import make_identity

B, N, D, E = 4, 2048, 512, 512
H, CH = 8, 64
HPC = 4              # heads per core
EC = HPC * CH        # 256 E-columns per core
SCALE = 1.0 / 8.0    # 1/sqrt(CH)
NT = N // 128        # 16 j-tiles
DT = D // 128        # 4 d-tiles

SIGMA = 0.055
A_SCH = float(np.float32(128.0 * np.log2(np.e) * SCALE))
B_SCH = float(np.float32(128.0 * (127.0 - SIGMA)))
# Schraudolph j-tiles per pass (none in pass 0: it is projection-bound;
# j=15 keeps the pass-boundary st WAR off the last ACT)
SCH_BY_PASS = [()] + [(1, 5, 14, 15)] * 7

F32 = mybir.dt.float32
BF16 = mybir.dt.bfloat16
I16 = mybir.dt.int16
NP_BF16 = ml_dtypes.bfloat16

_cache = {}


def _build():
    nc = bacc.Bacc("TRN2", target_bir_lowering=False, debug=False)

    # x tensors host-interleaved: row p = [c-major][t-major][n'] so a
    # 512-column chunk is 4KB contiguous per row (DMA packets are
    # overhead-bound, so line size is the bandwidth lever)
    xq = nc.dram_tensor("xq", [128, DT * N], BF16, kind="ExternalInput")
    xk = nc.dram_tensor("xk", [128, DT * N], BF16, kind="ExternalInput")
    xv = nc.dram_tensor("xv", [128, DT * N], BF16, kind="ExternalInput")
    wq = nc.dram_tensor("wq", [128, DT * EC], BF16, kind="ExternalInput")
    wk = nc.dram_tensor("wk", [128, DT * EC], BF16, kind="ExternalInput")
    wv = nc.dram_tensor("wv", [128, DT * EC], BF16, kind="ExternalInput")
    # all four bias vectors as one [128, 4] tensor (cols: bq m0, bq m1,
    # bk m0, bk m1) -- a [128,1] DMA degenerates to 4-byte packets
    bmat = nc.dram_tensor("bmat", [128, 4], F32, kind="ExternalInput")
    bvr = nc.dram_tensor("bvr", [128, EC], F32, kind="ExternalInput")
    # output blocks land contiguous per (head, chunk); host reassembles
    out = nc.dram_tensor("out", [HPC * 4 * 128, 256], F32, kind="ExternalOutput")

    with tile.TileContext(nc) as tc:
        with (
            tc.tile_pool(name="singles", bufs=1) as singles,
            tc.tile_pool(name="qkv", bufs=1) as qkv,
            tc.tile_pool(name="fin", bufs=3) as fin_pool,
        ):
            # ---- SBUF staging ----
            dummy = singles.tile([128, 512], BF16, tag="dummy", name="dummy")
            # flat, chunk-major (c, t, n') so every chunk DMA is a 2D copy
            # with 4KB contiguous per partition (max DMA packet size)
            xq_sb = singles.tile([128, DT * N], BF16, tag="xq", name="xq")
            xk_sb = singles.tile([128, DT * N], BF16, tag="xk", name="xk")
            xv_sb = singles.tile([128, DT * N], BF16, tag="xv", name="xv")
            wq_sb = singles.tile([128, DT * EC], BF16, tag="wq", name="wq")
            wk_sb = singles.tile([128, DT * EC], BF16, tag="wk", name="wk")
            wv_sb = singles.tile([128, DT * EC], BF16, tag="wv", name="wv")
            bm_sb = singles.tile([128, 4], F32, tag="bm", name="bm")
            bq_sb = [bm_sb[:, m:m + 1] for m in range(2)]
            bk_sb = [bm_sb[:, 2 + m:3 + m] for m in range(2)]
            bvr_sb = singles.tile([128, EC], F32, tag="bvr", name="bvr")
            ident = singles.tile([65, 65], F32, tag="ident", name="ident")

            # ---- engine warm-up (emitted first on their queues) ----
            nc.vector.memset(dummy, 0.0)
            gate_sb = singles.tile([1, 8], BF16, tag="gate", name="gate")

            # ---- input DMAs: merged descriptors, critical-first ----
            def xq_c(c):
                return (xq_sb[:, c * 2048:(c + 1) * 2048],
                        xq[:, c * 2048:(c + 1) * 2048])

            def xk_c(c):
                return (xk_sb[:, c * 2048:(c + 1) * 2048],
                        xk[:, c * 2048:(c + 1) * 2048])

            def xv_c(c):
                return (xv_sb[:, c * 2048:(c + 1) * 2048],
                        xv[:, c * 2048:(c + 1) * 2048])

            def cview(sb):  # [128, 4c*4t*512] -> [128, c, t, n']
                return sb.rearrange("p (c t n) -> p c t n", c=4, t=DT)

            def wview(sb):  # [128, 4t*EC] -> [128, t, e]
                return sb.rearrange("p (t e) -> p t e", t=DT)

            # wave 1 (ungated): QK projection critical path + V pass-0 needs.
            # Everything else is gated behind wave-1 arrival by gpsimd
            # *compute* ops (a DMA's sem-wait rides the descriptor, so a
            # gating DMA would not block later queue entries -- a tensor_copy
            # does).
            nc.sync.dma_start(wq_sb, wq[:, :])
            nc.sync.dma_start(*xq_c(0))
            nc.scalar.dma_start(bm_sb, bmat[:, :])
            nc.scalar.dma_start(wk_sb, wk[:, :])
            nc.scalar.dma_start(*xk_c(0))
            # V wave gated on xk c0 so the K critical path gets full DMA bw
            nc.vector.tensor_copy(wv_sb[0:1, 0:2], xk_sb[0:1, 0:2])
            nc.vector.tensor_copy(xv_sb[0:1, 0:2], xk_sb[0:1, 0:2])
            nc.vector.tensor_copy(bvr_sb[0:1, 0:2], xk_sb[0:1, 0:4].bitcast(F32))
            nc.gpsimd.dma_start(wv_sb, wv[:, :])
            nc.gpsimd.dma_start(*xv_c(0))
            nc.gpsimd.dma_start(bvr_sb, bvr[:, :])
            # ACT table preload, after scalar's DMA issues
            actwarm = singles.tile([1, 8], BF16, tag="actwarm", name="actwarm")
            nc.scalar.activation(
                actwarm, dummy[0:1, 0:8], mybir.ActivationFunctionType.Exp,
                scale=SCALE,
            )
            # wave 2: WAW-gated on xq c0 arrival (tiny pre-write into the
            # DMA destination forces the DMA to wait; emission-order gating
            # does not survive the scheduler)
            nc.vector.tensor_copy(xk_sb[0:1, 2048:2050], xq_sb[0:1, 0:2])
            nc.gpsimd.dma_start(*xk_c(1))

            # ---- working tiles ----
            qt_sb = [qkv.tile([128, N], BF16, tag=f"qt{m}", name=f"qt{m}") for m in range(2)]
            kt_sb = [qkv.tile([128, N], BF16, tag=f"kt{m}", name=f"kt{m}") for m in range(2)]
            v_sb = [qkv.tile([128, HPC * 65], BF16, tag=f"v{t}", name=f"v{t}") for t in range(NT)]
            for t in range(NT):
                ones_view = v_sb[t].rearrange("p (h c) -> p h c", c=65)[:, :, 64:65]
                nc.vector.memset(ones_view, 1.0)
            ots_sb = [qkv.tile([65, N], F32, tag=f"ots{h}", name=f"ots{h}") for h in range(HPC)]
            make_identity(nc, ident)

            with (
                tc.tile_pool(name="proj_ps", bufs=2, space="PSUM") as proj_ps,
                tc.tile_pool(name="st_ps", bufs=2, space="PSUM") as st_ps,
                tc.tile_pool(name="ot_ps", bufs=1, space="PSUM") as ot_ps,
                tc.tile_pool(name="pt_sb", bufs=4) as pt_pool,
                tc.tile_pool(name="pti_sb", bufs=2) as pti_pool,
            ):
                # PE warm-up: ~14 dummy matmuls flip HAM to 8/8 during DMA
                for i in range(10):
                    ps = proj_ps.tile([128, 512], F32, tag="proj", name="warm")
                    nc.tensor.matmul(ps, lhsT=dummy[:, 0:128], rhs=dummy,
                                     start=True, stop=True)

                # -- emitters --
                def emit_qk_group(dst, w_s, x_s, b_s, hp, nch):
                    ps = proj_ps.tile([128, 512], F32, tag="proj", name="qkp")
                    for t in range(DT):
                        nc.tensor.matmul(
                            ps,
                            lhsT=wview(w_s)[:, t, hp * 128:(hp + 1) * 128],
                            rhs=cview(x_s)[:, nch, t, :],
                            start=(t == 0),
                            stop=(t == DT - 1),
                        )
                    nc.vector.tensor_scalar_add(
                        dst[hp][:, nch * 512:(nch + 1) * 512], ps, b_s[hp]
                    )

                def emit_v_group(t):
                    ps = proj_ps.tile([128, EC], F32, tag="proj", name="vp")
                    for d in range(DT):
                        nc.tensor.matmul(
                            ps,
                            lhsT=cview(xv_sb)[:, t // 4, d,
                                              (t % 4) * 128:(t % 4 + 1) * 128],
                            rhs=wview(wv_sb)[:, d, :],
                            start=(d == 0),
                            stop=(d == DT - 1),
                        )
                    v_view = v_sb[t].rearrange("p (h c) -> p h c", c=65)[:, :, 0:64]
                    nc.vector.tensor_add(
                        v_view,
                        ps.rearrange("p (h c) -> p h c", c=64),
                        bvr_sb.rearrange("p (h c) -> p h c", c=64),
                    )

                def emit_filler(f):
                    if f[0] == "v":
                        emit_v_group(f[1])
                    elif f[0] == "q":
                        emit_qk_group(qt_sb, wq_sb, xq_sb, bq_sb, f[1], f[2])
                    else:
                        emit_qk_group(kt_sb, wk_sb, xk_sb, bk_sb, f[1], f[2])

                def emit_s_pair(hp, icol, j):
                    st = st_ps.tile([128, 1024], F32, tag="st", name="st")
                    for half in range(2):
                        ho = half * 64
                        nc.tensor.matmul(
                            st[:, half * 512:(half + 1) * 512],
                            lhsT=kt_sb[hp][ho:ho + 64, j * 128:(j + 1) * 128],
                            rhs=qt_sb[hp][ho:ho + 64, icol:icol + 512],
                            start=True,
                            stop=True,
                        )
                    return st

                ob_state = {}

                def emit_out_tr(hd, c, t):
                    # one transpose step of head hd's OT chunk c
                    if t == 0:
                        ob_state[(hd, c)] = proj_ps.tile(
                            [128, 512], F32, tag="proj", name="tr"
                        )
                    tr = ob_state[(hd, c)]
                    nc.tensor.transpose(
                        tr[:, t * 65:(t + 1) * 65],
                        ots_sb[hd][:, (c * 4 + t) * 128:(c * 4 + t + 1) * 128],
                        ident,
                    )

                def emit_out_fin(hd, c):
                    # batched recip, 4 muls, one DMA for the finished group
                    tr = ob_state.pop((hd, c))
                    tr3 = tr[:, 0:260].rearrange("p (t c) -> p t c", c=65)
                    rec = fin_pool.tile([128, 4], F32, tag="rec", name="rec")
                    nc.vector.reciprocal(rec, tr3[:, :, 64])
                    otile = fin_pool.tile([128, 256], F32, tag="otile", name="otile")
                    for t in range(4):
                        nc.vector.tensor_scalar_mul(
                            otile[:, t * 64:(t + 1) * 64],
                            tr3[:, t, 0:64],
                            rec[:, t:t + 1],
                        )
                    blk = (hd * 4 + c) * 128
                    eng = nc.sync if (hd + c) % 2 == 0 else nc.gpsimd
                    eng.dma_start(out[blk:blk + 128, :], otile)

                # filler schedule: (pass, iter) -> list of jobs
                fillers = {
                    (0, 0): [("v", 0)], (0, 1): [("k", 0, 1), ("v", 1)],
                    (0, 2): [("v", 2), ("v", 3)], (0, 3): [("v", 4)],
                    (0, 4): [("v", 5), ("v", 6)], (0, 5): [("k", 0, 2)],
                    (0, 6): [("v", 7), ("v", 8)], (0, 7): [("v", 9)],
                    (0, 8): [("v", 10), ("v", 11)], (0, 9): [("k", 0, 3)],
                    (0, 10): [("v", 12), ("v", 13)],
                    (0, 11): [("q", 0, 1)],
                    (0, 12): [("v", 14)], (0, 13): [("v", 15)],
                    (1, 7): [("q", 0, 2)], (1, 13): [("k", 1, 0)],
                    (1, 14): [("k", 1, 1)],
                    (2, 7): [("q", 0, 3)], (2, 13): [("k", 1, 2)],
                    (2, 14): [("k", 1, 3)],
                    (3, 7): [("q", 1, 0)],
                    (4, 7): [("q", 1, 1)],
                    (5, 7): [("q", 1, 2)],
                    (6, 7): [("q", 1, 3)],
                }
                # out-block steps: (pass, iter) -> ("tr", head, chunk, t)
                # or ("fin", head, chunk); transposes spread one per iter
                obsteps = {}
                for p in range(1, 8):
                    hp_prev, c_prev = (p - 1) // 4, (p - 1) % 4
                    for t in range(4):
                        obsteps[(p, 2 + t)] = ("tr", 2 * hp_prev, c_prev, t)
                        obsteps[(p, 8 + t)] = ("tr", 2 * hp_prev + 1, c_prev, t)
                    obsteps[(p, 6)] = ("fin", 2 * hp_prev, c_prev)
                    obsteps[(p, 12)] = ("fin", 2 * hp_prev + 1, c_prev)

                # -- prologue for pass 0 --
                emit_qk_group(qt_sb, wq_sb, xq_sb, bq_sb, 0, 0)
                emit_qk_group(kt_sb, wk_sb, xk_sb, bk_sb, 0, 0)

                # wave 3 inputs: WAW-gated on the first QT chunk
                qg = qt_sb[0][0:1, 0:2]
                for sb, cc in ((xk_sb, 2), (xv_sb, 1), (xk_sb, 3),
                               (xq_sb, 1), (xv_sb, 2), (xq_sb, 2),
                               (xv_sb, 3), (xq_sb, 3)):
                    nc.vector.tensor_copy(
                        sb[0:1, cc * 2048:cc * 2048 + 2], qg)
                nc.gpsimd.dma_start(*xk_c(2))
                nc.gpsimd.dma_start(*xv_c(1))
                nc.gpsimd.dma_start(*xk_c(3))
                nc.gpsimd.dma_start(*xq_c(1))
                nc.gpsimd.dma_start(*xv_c(2))
                nc.gpsimd.dma_start(*xq_c(2))
                nc.gpsimd.dma_start(*xv_c(3))
                nc.gpsimd.dma_start(*xq_c(3))

                prologue = [None, None]
                for p in range(8):
                    hp, c = p // 4, p % 4
                    icol = c * 512
                    ha, hb = 2 * hp, 2 * hp + 1
                    sch = SCH_BY_PASS[p]
                    ot = ot_ps.tile([65, 1024], F32, tag="ot", name="ot")
                    sts = [None] * NT
                    pts = [None] * NT

                    if p == 0:
                        sts[0] = emit_s_pair(hp, icol, 0)
                        sts[1] = emit_s_pair(hp, icol, 1)
                    else:
                        sts[0], sts[1] = prologue

                    def emit_av_pair(j):
                        for half, hd in ((0, ha), (1, hb)):
                            nc.tensor.matmul(
                                ot[:, half * 512:(half + 1) * 512],
                                lhsT=v_sb[j][:, hd * 65:(hd + 1) * 65],
                                rhs=pts[j][:, half * 512:(half + 1) * 512],
                                start=(j == 0),
                                stop=(j == NT - 1),
                            )

                    def emit_sch(j):
                        # Schraudolph exp on DVE, one iter ahead of its slot
                        # so the st-buffer WAR never stalls the S pipeline
                        pti = pti_pool.tile([128, 1024], I16, tag="pti", name="pti")
                        nc.vector.tensor_scalar(
                            pti, sts[j], A_SCH, B_SCH,
                            mybir.AluOpType.mult, mybir.AluOpType.add,
                        )
                        pts[j] = pti.bitcast(BF16)

                    if 0 in sch:
                        emit_sch(0)
                    for j in range(NT):
                        if j not in sch:
                            pt = pt_pool.tile([128, 1024], BF16, tag="pt", name="pt")
                            nc.scalar.activation(
                                pt, sts[j], mybir.ActivationFunctionType.Exp,
                                scale=SCALE,
                            )
                            pts[j] = pt
                        if j + 1 in sch:
                            emit_sch(j + 1)
                        if j % 2 == 1:
                            if j >= 3:
                                emit_av_pair(j - 3)
                            if j >= 2:
                                emit_av_pair(j - 2)
                        if j % 2 == 0:
                            for jj in (j + 2, j + 3):
                                if jj < NT:
                                    sts[jj] = emit_s_pair(hp, icol, jj)
                                elif p + 1 < 8:
                                    nhp, nc_ = (p + 1) // 4, (p + 1) % 4
                                    prologue[jj - NT] = emit_s_pair(
                                        nhp, nc_ * 512, jj - NT)
                        for f in fillers.get((p, j), ()):
                            emit_filler(f)
                        step = obsteps.get((p, j))
                        if step is not None:
                            if step[0] == "tr":
                                emit_out_tr(step[1], step[2], step[3])
                            else:
                                emit_out_fin(step[1], step[2])

                    emit_av_pair(NT - 2)
                    emit_av_pair(NT - 1)
                    nc.vector.tensor_copy(ots_sb[ha][:, icol:icol + 512], ot[:, 0:512])
                    nc.vector.tensor_copy(ots_sb[hb][:, icol:icol + 512], ot[:, 512:1024])

                # tail: last pair's final chunk
                for t in range(4):
                    emit_out_tr(2, 3, t)
                emit_out_fin(2, 3)
                for t in range(4):
                    emit_out_tr(3, 3, t)
                emit_out_fin(3, 3)

    nc.compile()
    return nc


def _get_nc():
    if "nc" not in _cache:
        _cache["nc"] = _build()
    return _cache["nc"]


def _ilv_x(xT):
    # [D, N] -> [128, 4c * 4t * 512n'] with row p = [c][t][n'] interleave
    return np.ascontiguousarray(
        xT.reshape(DT, 128, 4, 512).transpose(1, 2, 0, 3).reshape(128, DT * N)
    ).astype(NP_BF16)


def _ilv_w(w):
    # [D, EC] -> [128, 4t * EC]
    return np.ascontiguousarray(
        w.reshape(DT, 128, EC).transpose(1, 0, 2).reshape(128, DT * EC)
    ).astype(NP_BF16)


def _shard_inputs(q, k, v, Wq, Wk, Wv, bq, bk, bv):
    in_maps = []
    for c in range(8):
        b, g = c // 2, c % 2
        sl = slice(g * EC, (g + 1) * EC)
        in_maps.append({
            "xq": _ilv_x(np.asarray(q)[b].T),
            "xk": _ilv_x(np.asarray(k)[b].T),
            "xv": _ilv_x(np.asarray(v)[b].T),
            "wq": _ilv_w(np.asarray(Wq)[:, sl]),
            "wk": _ilv_w(np.asarray(Wk)[:, sl]),
            "wv": _ilv_w(np.asarray(Wv)[:, sl]),
            "bmat": np.stack([
                np.asarray(bq)[sl][0:128], np.asarray(bq)[sl][128:256],
                np.asarray(bk)[sl][0:128], np.asarray(bk)[sl][128:256],
            ], axis=1).astype(np.float32),
            "bvr": np.ascontiguousarray(
                np.broadcast_to(np.asarray(bv)[sl], (128, EC))
            ).astype(np.float32),
        })
    return in_maps


def kernel(q, k, v, Wq, Wk, Wv, bq, bk, bv, _trace=False):
    nc = _get_nc()
    in_maps = _shard_inputs(q, k, v, Wq, Wk, Wv, bq, bk, bv)
    res = run_bass_kernel_spmd(
        nc, in_maps, core_ids=list(range(8)), trace=_trace
    )
    out = np.empty((B, N, E), np.float32)
    for c in range(8):
        b, g = c // 2, c % 2
        o2 = res.results[c]["out"].reshape(HPC, 4, 128, 4, 64)
        out[b, :, g * EC:(g + 1) * EC] = (
            o2.transpose(1, 3, 2, 0, 4).reshape(N, EC)
        )
    if _trace:
        _cache["last_exec_time_ns"] = res.exec_time_ns
    return out


# revision 31
# speedup vs baseline: 1.0184x; 1.0088x over previous
"""Multi-head attention TRN2 Bass kernel (v2).

Problem: B=4, N=2048, D=E=512, 8 heads (ch=64).
out = softmax((x_q Wq + bq)(x_k Wk + bk)^T / 8) (x_v Wv + bv), per head.

Sharding (8 cores): core c handles batch b = c//2 and head-group g = c%2
(4 heads = 256 E-columns). Each core is fully independent (no collectives).

v2 changes over the original ACT-paced design:
  - Pass = (head-pair, i-chunk of 512). The two heads of a pair occupy
    SBUF partitions 0-63 / 64-127 of QT/KT, so their S^T matmuls issue as
    back-to-back row-tiled pairs (tile_position (0,0)/(64,0)) that execute
    CONCURRENTLY on the PE (HW-probed: 113 ns/MM vs 215 serial, 1.9x).
  - Part of the exp work moves off the ACT engine onto the DVE as a
    Schraudolph bit-trick: P_bf16bits = int16(rint(A*S + B)), one
    tensor_scalar (fp32 PSUM -> int16 SBUF, round-to-nearest verified on
    HW), bitcast to bf16 for the AV matmul. Host-simulated rel-err with
    this split: ~0.012 (gate 0.02).
  - Input DMAs are merged into few large 3D descriptors, spread across
    sync/vector/scalar/gpsimd queues, issued critical-first (wq+xq first).
  - PE warm-up dummies + early exp-table preload hide the HAM cold clock
    (4/8 = 1.2 GHz) and the 2.7us ACT table load during the input DMA.
  - Output blocks are staged 4-at-a-time in SBUF and written with one DMA
    per (head, 512-chunk): 16 output DMAs instead of 64.
"""

import numpy as np
import ml_dtypes

import concourse.bacc as bacc
import concourse.mybir as mybir
import concourse.tile as tile
from concourse.bass_utils import run_bass_kernel_spmd
from concourse.masks import make_identity

B, N, D, E = 4, 2048, 512, 512
H, CH = 8, 64
HPC = 4              # heads per core
EC = HPC * CH        # 256 E-columns per core
SCALE = 1.0 / 8.0    # 1/sqrt(CH)
NT = N // 128        # 16 j-tiles
DT = D // 128        # 4 d-tiles

SIGMA = 0.055
A_SCH = float(np.float32(128.0 * np.log2(np.e) * SCALE))
B_SCH = float(np.float32(128.0 * (127.0 - SIGMA)))
# Schraudolph j-tiles per pass (none in pass 0: it is projection-bound;
# j=15 keeps the pass-boundary st WAR off the last ACT)
SCH_BY_PASS = [()] + [(2, 5, 9, 12)] * 7

F32 = mybir.dt.float32
BF16 = mybir.dt.bfloat16
I16 = mybir.dt.int16
NP_BF16 = ml_dtypes.bfloat16

_cache = {}


def _build():
    nc = bacc.Bacc("TRN2", target_bir_lowering=False, debug=False)

    # x tensors host-interleaved: row p = [c-major][t-major][n'] so a
    # 512-column chunk is 4KB contiguous per row (DMA packets are
    # overhead-bound, so line size is the bandwidth lever)
    xq = nc.dram_tensor("xq", [128, DT * N], BF16, kind="ExternalInput")
    xk = nc.dram_tensor("xk", [128, DT * N], BF16, kind="ExternalInput")
    xv = nc.dram_tensor("xv", [128, DT * N], BF16, kind="ExternalInput")
    wq = nc.dram_tensor("wq", [128, DT * EC], BF16, kind="ExternalInput")
    wk = nc.dram_tensor("wk", [128, DT * EC], BF16, kind="ExternalInput")
    wv = nc.dram_tensor("wv", [128, DT * EC], BF16, kind="ExternalInput")
    # all four bias vectors as one [128, 4] tensor (cols: bq m0, bq m1,
    # bk m0, bk m1) -- a [128,1] DMA degenerates to 4-byte packets
    bmat = nc.dram_tensor("bmat", [128, 4], F32, kind="ExternalInput")
    bvr = nc.dram_tensor("bvr", [128, EC], F32, kind="ExternalInput")
    # output blocks land contiguous per (head, chunk); host reassembles
    out = nc.dram_tensor("out", [HPC * 4 * 128, 256], F32, kind="ExternalOutput")

    with tile.TileContext(nc) as tc:
        with (
            tc.tile_pool(name="singles", bufs=1) as singles,
            tc.tile_pool(name="qkv", bufs=1) as qkv,
            tc.tile_pool(name="fin", bufs=3) as fin_pool,
        ):
            # ---- SBUF staging ----
            dummy = singles.tile([128, 512], BF16, tag="dummy", name="dummy")
            # flat, chunk-major (c, t, n') so every chunk DMA is a 2D copy
            # with 4KB contiguous per partition (max DMA packet size)
            xq_sb = singles.tile([128, DT * N], BF16, tag="xq", name="xq")
            xk_sb = singles.tile([128, DT * N], BF16, tag="xk", name="xk")
            xv_sb = singles.tile([128, DT * N], BF16, tag="xv", name="xv")
            wq_sb = singles.tile([128, DT * EC], BF16, tag="wq", name="wq")
            wk_sb = singles.tile([128, DT * EC], BF16, tag="wk", name="wk")
            wv_sb = singles.tile([128, DT * EC], BF16, tag="wv", name="wv")
            bm_sb = singles.tile([128, 4], F32, tag="bm", name="bm")
            bq_sb = [bm_sb[:, m:m + 1] for m in range(2)]
            bk_sb = [bm_sb[:, 2 + m:3 + m] for m in range(2)]
            bvr_sb = singles.tile([128, EC], F32, tag="bvr", name="bvr")
            ident = singles.tile([65, 65], F32, tag="ident", name="ident")

            # ---- engine warm-up (emitted first on their queues) ----
            nc.vector.memset(dummy, 0.0)
            gate_sb = singles.tile([1, 8], BF16, tag="gate", name="gate")

            # ---- input DMAs: merged descriptors, critical-first ----
            def xq_c(c):
                return (xq_sb[:, c * 2048:(c + 1) * 2048],
                        xq[:, c * 2048:(c + 1) * 2048])

            def xk_c(c):
                return (xk_sb[:, c * 2048:(c + 1) * 2048],
                        xk[:, c * 2048:(c + 1) * 2048])

            def xv_c(c):
                return (xv_sb[:, c * 2048:(c + 1) * 2048],
                        xv[:, c * 2048:(c + 1) * 2048])

            def cview(sb):  # [128, 4c*4t*512] -> [128, c, t, n']
                return sb.rearrange("p (c t n) -> p c t n", c=4, t=DT)

            def wview(sb):  # [128, 4t*EC] -> [128, t, e]
                return sb.rearrange("p (t e) -> p t e", t=DT)

            # wave 1 (ungated): QK projection critical path + V pass-0 needs.
            # Everything else is gated behind wave-1 arrival by gpsimd
            # *compute* ops (a DMA's sem-wait rides the descriptor, so a
            # gating DMA would not block later queue entries -- a tensor_copy
            # does).
            nc.sync.dma_start(wq_sb, wq[:, :])
            nc.sync.dma_start(*xq_c(0))
            nc.scalar.dma_start(bm_sb, bmat[:, :])
            nc.scalar.dma_start(wk_sb, wk[:, :])
            nc.scalar.dma_start(*xk_c(0))
            # V wave gated on xk c0 so the K critical path gets full DMA bw
            nc.vector.tensor_copy(wv_sb[0:1, 0:2], xk_sb[0:1, 0:2])
            nc.vector.tensor_copy(xv_sb[0:1, 0:2], xk_sb[0:1, 0:2])
            nc.vector.tensor_copy(bvr_sb[0:1, 0:2], xk_sb[0:1, 0:4].bitcast(F32))
            nc.gpsimd.dma_start(wv_sb, wv[:, :])
            nc.gpsimd.dma_start(*xv_c(0))
            nc.gpsimd.dma_start(bvr_sb, bvr[:, :])
            # ACT table preload, after scalar's DMA issues
            actwarm = singles.tile([1, 8], BF16, tag="actwarm", name="actwarm")
            nc.scalar.activation(
                actwarm, dummy[0:1, 0:8], mybir.ActivationFunctionType.Exp,
                scale=SCALE,
            )
            # wave 2: WAW-gated on xq c0 arrival (tiny pre-write into the
            # DMA destination forces the DMA to wait; emission-order gating
            # does not survive the scheduler)
            nc.vector.tensor_copy(xk_sb[0:1, 2048:2050], xq_sb[0:1, 0:2])
            nc.gpsimd.dma_start(*xk_c(1))

            # ---- working tiles ----
            qt_sb = [qkv.tile([128, N], BF16, tag=f"qt{m}", name=f"qt{m}") for m in range(2)]
            kt_sb = [qkv.tile([128, N], BF16, tag=f"kt{m}", name=f"kt{m}") for m in range(2)]
            v_sb = [qkv.tile([128, HPC * 65], BF16, tag=f"v{t}", name=f"v{t}") for t in range(NT)]
            for t in range(NT):
                ones_view = v_sb[t].rearrange("p (h c) -> p h c", c=65)[:, :, 64:65]
                nc.vector.memset(ones_view, 1.0)
            ots_sb = [qkv.tile([65, N], F32, tag=f"ots{h}", name=f"ots{h}") for h in range(HPC)]
            make_identity(nc, ident)

            with (
                tc.tile_pool(name="proj_ps", bufs=2, space="PSUM") as proj_ps,
                tc.tile_pool(name="st_ps", bufs=2, space="PSUM") as st_ps,
                tc.tile_pool(name="ot_ps", bufs=1, space="PSUM") as ot_ps,
                tc.tile_pool(name="pt_sb", bufs=6) as pt_pool,
                tc.tile_pool(name="pti_sb", bufs=4) as pti_pool,
            ):
                # PE warm-up: ~14 dummy matmuls flip HAM to 8/8 during DMA
                for i in range(10):
                    ps = proj_ps.tile([128, 512], F32, tag="proj", name="warm")
                    nc.tensor.matmul(ps, lhsT=dummy[:, 0:128], rhs=dummy,
                                     start=True, stop=True)

                # -- emitters --
                def emit_qk_group(dst, w_s, x_s, b_s, hp, nch):
                    ps = proj_ps.tile([128, 512], F32, tag="proj", name="qkp")
                    for t in range(DT):
                        nc.tensor.matmul(
                            ps,
                            lhsT=wview(w_s)[:, t, hp * 128:(hp + 1) * 128],
                            rhs=cview(x_s)[:, nch, t, :],
                            start=(t == 0),
                            stop=(t == DT - 1),
                        )
                    nc.vector.tensor_scalar_add(
                        dst[hp][:, nch * 512:(nch + 1) * 512], ps, b_s[hp]
                    )

                def emit_v_group(t):
                    ps = proj_ps.tile([128, EC], F32, tag="proj", name="vp")
                    for d in range(DT):
                        nc.tensor.matmul(
                            ps,
                            lhsT=cview(xv_sb)[:, t // 4, d,
                                              (t % 4) * 128:(t % 4 + 1) * 128],
                            rhs=wview(wv_sb)[:, d, :],
                            start=(d == 0),
                            stop=(d == DT - 1),
                        )
                    v_view = v_sb[t].rearrange("p (h c) -> p h c", c=65)[:, :, 0:64]
                    nc.vector.tensor_add(
                        v_view,
                        ps.rearrange("p (h c) -> p h c", c=64),
                        bvr_sb.rearrange("p (h c) -> p h c", c=64),
                    )

                def emit_filler(f):
                    if f[0] == "v":
                        emit_v_group(f[1])
                    elif f[0] == "q":
                        emit_qk_group(qt_sb, wq_sb, xq_sb, bq_sb, f[1], f[2])
                    else:
                        emit_qk_group(kt_sb, wk_sb, xk_sb, bk_sb, f[1], f[2])

                def emit_s_pair(hp, icol, j):
                    st = st_ps.tile([128, 1024], F32, tag="st", name="st")
                    for half in range(2):
                        ho = half * 64
                        nc.tensor.matmul(
                            st[:, half * 512:(half + 1) * 512],
                            lhsT=kt_sb[hp][ho:ho + 64, j * 128:(j + 1) * 128],
                            rhs=qt_sb[hp][ho:ho + 64, icol:icol + 512],
                            start=True,
                            stop=True,
                        )
                    return st

                ob_state = {}

                def emit_out_tr(hd, c, t):
                    # one transpose step of head hd's OT chunk c
                    if t == 0:
                        ob_state[(hd, c)] = proj_ps.tile(
                            [128, 512], F32, tag="proj", name="tr"
                        )
                    tr = ob_state[(hd, c)]
                    nc.tensor.transpose(
                        tr[:, t * 65:(t + 1) * 65],
                        ots_sb[hd][:, (c * 4 + t) * 128:(c * 4 + t + 1) * 128],
                        ident,
                    )

                def emit_out_fin(hd, c):
                    # batched recip, 4 muls, one DMA for the finished group
                    tr = ob_state.pop((hd, c))
                    tr3 = tr[:, 0:260].rearrange("p (t c) -> p t c", c=65)
                    rec = fin_pool.tile([128, 4], F32, tag="rec", name="rec")
                    nc.vector.reciprocal(rec, tr3[:, :, 64])
                    otile = fin_pool.tile([128, 256], F32, tag="otile", name="otile")
                    for t in range(4):
                        nc.vector.tensor_scalar_mul(
                            otile[:, t * 64:(t + 1) * 64],
                            tr3[:, t, 0:64],
                            rec[:, t:t + 1],
                        )
                    blk = (hd * 4 + c) * 128
                    eng = nc.sync if (hd + c) % 2 == 0 else nc.gpsimd
                    eng.dma_start(out[blk:blk + 128, :], otile)

                # filler schedule: (pass, iter) -> list of jobs
                fillers = {
                    (0, 0): [("v", 0)], (0, 1): [("k", 0, 1), ("v", 1)],
                    (0, 2): [("v", 2), ("v", 3)], (0, 3): [("v", 4)],
                    (0, 4): [("v", 5), ("v", 6)], (0, 5): [("k", 0, 2)],
                    (0, 6): [("v", 7), ("v", 8)], (0, 7): [("v", 9)],
                    (0, 8): [("v", 10), ("v", 11)], (0, 9): [("k", 0, 3)],
                    (0, 10): [("v", 12), ("v", 13)],
                    (0, 11): [("q", 0, 1)],
                    (0, 12): [("v", 14)], (0, 13): [("v", 15)],
                    (1, 7): [("q", 0, 2)], (1, 13): [("k", 1, 0)],
                    (1, 14): [("k", 1, 1)],
                    (2, 7): [("q", 0, 3)], (2, 13): [("k", 1, 2)],
                    (2, 14): [("k", 1, 3)],
                    (3, 7): [("q", 1, 0)],
                    (4, 7): [("q", 1, 1)],
                    (5, 7): [("q", 1, 2)],
                    (6, 7): [("q", 1, 3)],
                }
                # out-block steps: (pass, iter) -> ("tr", head, chunk, t)
                # or ("fin", head, chunk); transposes spread one per iter
                obsteps = {}
                for p in range(1, 8):
                    hp_prev, c_prev = (p - 1) // 4, (p - 1) % 4
                    for t in range(4):
                        obsteps[(p, 2 + t)] = ("tr", 2 * hp_prev, c_prev, t)
                        obsteps[(p, 8 + t)] = ("tr", 2 * hp_prev + 1, c_prev, t)
                    obsteps[(p, 6)] = ("fin", 2 * hp_prev, c_prev)
                    obsteps[(p, 12)] = ("fin", 2 * hp_prev + 1, c_prev)

                # -- prologue for pass 0 --
                emit_qk_group(qt_sb, wq_sb, xq_sb, bq_sb, 0, 0)
                for i in range(3):
                    ps = proj_ps.tile([128, 512], F32, tag="proj", name="warm2")
                    nc.tensor.matmul(ps, lhsT=dummy[:, 0:128], rhs=dummy,
                                     start=True, stop=True)
                emit_qk_group(kt_sb, wk_sb, xk_sb, bk_sb, 0, 0)

                # wave 3 inputs: WAW-gated on the first QT chunk
                qg = qt_sb[0][0:1, 0:2]
                for sb, cc in ((xk_sb, 2), (xv_sb, 1), (xk_sb, 3),
                               (xq_sb, 1), (xv_sb, 2), (xq_sb, 2),
                               (xv_sb, 3), (xq_sb, 3)):
                    nc.vector.tensor_copy(
                        sb[0:1, cc * 2048:cc * 2048 + 2], qg)
                nc.gpsimd.dma_start(*xk_c(2))
                nc.gpsimd.dma_start(*xv_c(1))
                nc.gpsimd.dma_start(*xk_c(3))
                nc.gpsimd.dma_start(*xq_c(1))
                nc.gpsimd.dma_start(*xv_c(2))
                nc.gpsimd.dma_start(*xq_c(2))
                nc.gpsimd.dma_start(*xv_c(3))
                nc.gpsimd.dma_start(*xq_c(3))

                prologue = [None, None]
                for p in range(8):
                    hp, c = p // 4, p % 4
                    icol = c * 512
                    ha, hb = 2 * hp, 2 * hp + 1
                    sch = SCH_BY_PASS[p]
                    ot = ot_ps.tile([65, 1024], F32, tag="ot", name="ot")
                    sts = [None] * NT
                    pts = [None] * NT

                    if p == 0:
                        sts[0] = emit_s_pair(hp, icol, 0)
                        sts[1] = emit_s_pair(hp, icol, 1)
                    else:
                        sts[0], sts[1] = prologue

                    def emit_av_pair(j):
                        for half, hd in ((0, ha), (1, hb)):
                            nc.tensor.matmul(
                                ot[:, half * 512:(half + 1) * 512],
                                lhsT=v_sb[j][:, hd * 65:(hd + 1) * 65],
                                rhs=pts[j][:, half * 512:(half + 1) * 512],
                                start=(j == 0),
                                stop=(j == NT - 1),
                            )

                    def emit_sch(j):
                        # Schraudolph exp on DVE, one iter ahead of its slot
                        # so the st-buffer WAR never stalls the S pipeline
                        pti = pti_pool.tile([128, 1024], I16, tag="pti", name="pti")
                        nc.vector.tensor_scalar(
                            pti, sts[j], A_SCH, B_SCH,
                            mybir.AluOpType.mult, mybir.AluOpType.add,
                        )
                        pts[j] = pti.bitcast(BF16)

                    if 0 in sch:
                        emit_sch(0)
                    for j in range(NT):
                        if j not in sch:
                            pt = pt_pool.tile([128, 1024], BF16, tag="pt", name="pt")
                            nc.scalar.activation(
                                pt, sts[j], mybir.ActivationFunctionType.Exp,
                                scale=SCALE,
                            )
                            pts[j] = pt
                        if j + 1 in sch:
                            emit_sch(j + 1)
                        if j >= 2:
                            emit_av_pair(j - 2)
                        if j % 2 == 0:
                            for jj in (j + 2, j + 3):
                                if jj < NT:
                                    sts[jj] = emit_s_pair(hp, icol, jj)
                                elif p + 1 < 8:
                                    nhp, nc_ = (p + 1) // 4, (p + 1) % 4
                                    prologue[jj - NT] = emit_s_pair(
                                        nhp, nc_ * 512, jj - NT)
                        for f in fillers.get((p, j), ()):
                            emit_filler(f)
                        step = obsteps.get((p, j))
                        if step is not None:
                            if step[0] == "tr":
                                emit_out_tr(step[1], step[2], step[3])
                            else:
                                emit_out_fin(step[1], step[2])

                    emit_av_pair(NT - 2)
                    emit_av_pair(NT - 1)
                    nc.vector.tensor_copy(ots_sb[ha][:, icol:icol + 512], ot[:, 0:512])
                    nc.vector.tensor_copy(ots_sb[hb][:, icol:icol + 512], ot[:, 512:1024])

                # tail: last pair's final chunk
                for t in range(4):
                    emit_out_tr(2, 3, t)
                emit_out_fin(2, 3)
                for t in range(4):
                    emit_out_tr(3, 3, t)
                emit_out_fin(3, 3)

    nc.compile()
    return nc


def _get_nc():
    if "nc" not in _cache:
        _cache["nc"] = _build()
    return _cache["nc"]


def _ilv_x(xT):
    # [D, N] -> [128, 4c * 4t * 512n'] with row p = [c][t][n'] interleave
    return np.ascontiguousarray(
        xT.reshape(DT, 128, 4, 512).transpose(1, 2, 0, 3).reshape(128, DT * N)
    ).astype(NP_BF16)


def _ilv_w(w):
    # [D, EC] -> [128, 4t * EC]
    return np.ascontiguousarray(
        w.reshape(DT, 128, EC).transpose(1, 0, 2).reshape(128, DT * EC)
    ).astype(NP_BF16)


def _shard_inputs(q, k, v, Wq, Wk, Wv, bq, bk, bv):
    in_maps = []
    for c in range(8):
        b, g = c // 2, c % 2
        sl = slice(g * EC, (g + 1) * EC)
        in_maps.append({
            "xq": _ilv_x(np.asarray(q)[b].T),
            "xk": _ilv_x(np.asarray(k)[b].T),
            "xv": _ilv_x(np.asarray(v)[b].T),
            "wq": _ilv_w(np.asarray(Wq)[:, sl]),
            "wk": _ilv_w(np.asarray(Wk)[:, sl]),
            "wv": _ilv_w(np.asarray(Wv)[:, sl]),
            "bmat": np.stack([
                np.asarray(bq)[sl][0:128], np.asarray(bq)[sl][128:256],
                np.asarray(bk)[sl][0:128], np.asarray(bk)[sl][128:256],
            ], axis=1).astype(np.float32),
            "bvr": np.ascontiguousarray(
                np.broadcast_to(np.asarray(bv)[sl], (128, EC))
            ).astype(np.float32),
        })
    return in_maps


def kernel(q, k, v, Wq, Wk, Wv, bq, bk, bv, _trace=False):
    nc = _get_nc()
    in_maps = _shard_inputs(q, k, v, Wq, Wk, Wv, bq, bk, bv)
    res = run_bass_kernel_spmd(
        nc, in_maps, core_ids=list(range(8)), trace=_trace
    )
    out = np.empty((B, N, E), np.float32)
    for c in range(8):
        b, g = c // 2, c % 2
        o2 = res.results[c]["out"].reshape(HPC, 4, 128, 4, 64)
        out[b, :, g * EC:(g + 1) * EC] = (
            o2.transpose(1, 3, 2, 0, 4).reshape(N, EC)
        )
    if _trace:
        _cache["last_exec_time_ns"] = res.exec_time_ns
    return out


# revision 33
# speedup vs baseline: 1.0271x; 1.0085x over previous
"""Multi-head attention TRN2 Bass kernel (v2).

Problem: B=4, N=2048, D=E=512, 8 heads (ch=64).
out = softmax((x_q Wq + bq)(x_k Wk + bk)^T / 8) (x_v Wv + bv), per head.

Sharding (8 cores): core c handles batch b = c//2 and head-group g = c%2
(4 heads = 256 E-columns). Each core is fully independent (no collectives).

v2 changes over the original ACT-paced design:
  - Pass = (head-pair, i-chunk of 512). The two heads of a pair occupy
    SBUF partitions 0-63 / 64-127 of QT/KT, so their S^T matmuls issue as
    back-to-back row-tiled pairs (tile_position (0,0)/(64,0)) that execute
    CONCURRENTLY on the PE (HW-probed: 113 ns/MM vs 215 serial, 1.9x).
  - Part of the exp work moves off the ACT engine onto the DVE as a
    Schraudolph bit-trick: P_bf16bits = int16(rint(A*S + B)), one
    tensor_scalar (fp32 PSUM -> int16 SBUF, round-to-nearest verified on
    HW), bitcast to bf16 for the AV matmul. Host-simulated rel-err with
    this split: ~0.012 (gate 0.02).
  - Input DMAs are merged into few large 3D descriptors, spread across
    sync/vector/scalar/gpsimd queues, issued critical-first (wq+xq first).
  - PE warm-up dummies + early exp-table preload hide the HAM cold clock
    (4/8 = 1.2 GHz) and the 2.7us ACT table load during the input DMA.
  - Output blocks are staged 4-at-a-time in SBUF and written with one DMA
    per (head, 512-chunk): 16 output DMAs instead of 64.
"""

import numpy as np
import ml_dtypes

import concourse.bacc as bacc
import concourse.mybir as mybir
import concourse.tile as tile
from concourse.bass_utils import run_bass_kernel_spmd
from concourse.masks import make_identity

B, N, D, E = 4, 2048, 512, 512
H, CH = 8, 64
HPC = 4              # heads per core
EC = HPC * CH        # 256 E-columns per core
SCALE = 1.0 / 8.0    # 1/sqrt(CH)
NT = N // 128        # 16 j-tiles
DT = D // 128        # 4 d-tiles

SIGMA = 0.055
A_SCH = float(np.float32(128.0 * np.log2(np.e) * SCALE))
B_SCH = float(np.float32(128.0 * (127.0 - SIGMA)))
# Schraudolph j-tiles per pass (none in pass 0: it is projection-bound;
# j=15 keeps the pass-boundary st WAR off the last ACT)
SCH_BY_PASS = [()] + [(2, 5, 9, 12)] * 7

F32 = mybir.dt.float32
BF16 = mybir.dt.bfloat16
I16 = mybir.dt.int16
NP_BF16 = ml_dtypes.bfloat16

_cache = {}


def _build():
    nc = bacc.Bacc("TRN2", target_bir_lowering=False, debug=False)

    # x tensors host-interleaved: row p = [c-major][t-major][n'] so a
    # 512-column chunk is 4KB contiguous per row (DMA packets are
    # overhead-bound, so line size is the bandwidth lever)
    xq = nc.dram_tensor("xq", [128, DT * N], BF16, kind="ExternalInput")
    xk = nc.dram_tensor("xk", [128, DT * N], BF16, kind="ExternalInput")
    xv = nc.dram_tensor("xv", [128, DT * N], BF16, kind="ExternalInput")
    wq = nc.dram_tensor("wq", [128, DT * EC], BF16, kind="ExternalInput")
    wk = nc.dram_tensor("wk", [128, DT * EC], BF16, kind="ExternalInput")
    wv = nc.dram_tensor("wv", [128, DT * EC], BF16, kind="ExternalInput")
    # all four bias vectors as one [128, 4] tensor (cols: bq m0, bq m1,
    # bk m0, bk m1) -- a [128,1] DMA degenerates to 4-byte packets
    bmat = nc.dram_tensor("bmat", [128, 4], F32, kind="ExternalInput")
    bvr = nc.dram_tensor("bvr", [128, EC], F32, kind="ExternalInput")
    # output blocks land contiguous per (head, chunk); host reassembles
    out = nc.dram_tensor("out", [HPC * 4 * 128, 256], F32, kind="ExternalOutput")

    with tile.TileContext(nc) as tc:
        with (
            tc.tile_pool(name="singles", bufs=1) as singles,
            tc.tile_pool(name="qkv", bufs=1) as qkv,
            tc.tile_pool(name="fin", bufs=3) as fin_pool,
        ):
            # ---- SBUF staging ----
            dummy = singles.tile([128, 512], BF16, tag="dummy", name="dummy")
            # flat, chunk-major (c, t, n') so every chunk DMA is a 2D copy
            # with 4KB contiguous per partition (max DMA packet size)
            xq_sb = singles.tile([128, DT * N], BF16, tag="xq", name="xq")
            xk_sb = singles.tile([128, DT * N], BF16, tag="xk", name="xk")
            xv_sb = singles.tile([128, DT * N], BF16, tag="xv", name="xv")
            wq_sb = singles.tile([128, DT * EC], BF16, tag="wq", name="wq")
            wk_sb = singles.tile([128, DT * EC], BF16, tag="wk", name="wk")
            wv_sb = singles.tile([128, DT * EC], BF16, tag="wv", name="wv")
            bm_sb = singles.tile([128, 4], F32, tag="bm", name="bm")
            bq_sb = [bm_sb[:, m:m + 1] for m in range(2)]
            bk_sb = [bm_sb[:, 2 + m:3 + m] for m in range(2)]
            bvr_sb = singles.tile([128, EC], F32, tag="bvr", name="bvr")
            ident = singles.tile([65, 65], F32, tag="ident", name="ident")

            # ---- engine warm-up (emitted first on their queues) ----
            nc.vector.memset(dummy, 0.0)
            gate_sb = singles.tile([1, 8], BF16, tag="gate", name="gate")

            # ---- input DMAs: merged descriptors, critical-first ----
            def xq_c(c):
                return (xq_sb[:, c * 2048:(c + 1) * 2048],
                        xq[:, c * 2048:(c + 1) * 2048])

            def xk_c(c):
                return (xk_sb[:, c * 2048:(c + 1) * 2048],
                        xk[:, c * 2048:(c + 1) * 2048])

            def xv_c(c):
                return (xv_sb[:, c * 2048:(c + 1) * 2048],
                        xv[:, c * 2048:(c + 1) * 2048])

            def cview(sb):  # [128, 4c*4t*512] -> [128, c, t, n']
                return sb.rearrange("p (c t n) -> p c t n", c=4, t=DT)

            def wview(sb):  # [128, 4t*EC] -> [128, t, e]
                return sb.rearrange("p (t e) -> p t e", t=DT)

            # wave 1 (ungated): QK projection critical path + V pass-0 needs.
            # Everything else is gated behind wave-1 arrival by gpsimd
            # *compute* ops (a DMA's sem-wait rides the descriptor, so a
            # gating DMA would not block later queue entries -- a tensor_copy
            # does).
            nc.sync.dma_start(wq_sb, wq[:, :])
            nc.sync.dma_start(*xq_c(0))
            nc.scalar.dma_start(bm_sb, bmat[:, :])
            nc.scalar.dma_start(wk_sb, wk[:, :])
            nc.scalar.dma_start(*xk_c(0))
            # V wave gated on xk c0 so the K critical path gets full DMA bw
            nc.vector.tensor_copy(wv_sb[0:1, 0:2], xk_sb[0:1, 0:2])
            nc.vector.tensor_copy(xv_sb[0:1, 0:2], xk_sb[0:1, 0:2])
            nc.vector.tensor_copy(bvr_sb[0:1, 0:2], xk_sb[0:1, 0:4].bitcast(F32))
            nc.gpsimd.dma_start(wv_sb, wv[:, :])
            nc.gpsimd.dma_start(*xv_c(0))
            nc.gpsimd.dma_start(bvr_sb, bvr[:, :])
            # ACT table preload, after scalar's DMA issues
            actwarm = singles.tile([1, 8], BF16, tag="actwarm", name="actwarm")
            nc.scalar.activation(
                actwarm, dummy[0:1, 0:8], mybir.ActivationFunctionType.Exp,
                scale=SCALE,
            )
            # wave 2: WAW-gated on xq c0 arrival (tiny pre-write into the
            # DMA destination forces the DMA to wait; emission-order gating
            # does not survive the scheduler)
            nc.vector.tensor_copy(xk_sb[0:1, 2048:2050], xq_sb[0:1, 0:2])
            nc.gpsimd.dma_start(*xk_c(1))

            # ---- working tiles ----
            qt_sb = [qkv.tile([128, N], BF16, tag=f"qt{m}", name=f"qt{m}") for m in range(2)]
            kt_sb = [qkv.tile([128, N], BF16, tag=f"kt{m}", name=f"kt{m}") for m in range(2)]
            v_sb = [qkv.tile([128, HPC * 65], BF16, tag=f"v{t}", name=f"v{t}") for t in range(NT)]
            for t in range(NT):
                ones_view = v_sb[t].rearrange("p (h c) -> p h c", c=65)[:, :, 64:65]
                nc.vector.memset(ones_view, 1.0)
            ots_sb = [qkv.tile([65, N], F32, tag=f"ots{h}", name=f"ots{h}") for h in range(HPC)]
            make_identity(nc, ident)

            with (
                tc.tile_pool(name="proj_ps", bufs=2, space="PSUM") as proj_ps,
                tc.tile_pool(name="st_ps", bufs=2, space="PSUM") as st_ps,
                tc.tile_pool(name="ot_ps", bufs=1, space="PSUM") as ot_ps,
                tc.tile_pool(name="pt_sb", bufs=6) as pt_pool,
                tc.tile_pool(name="pti_sb", bufs=4) as pti_pool,
            ):
                # PE warm-up: ~14 dummy matmuls flip HAM to 8/8 during DMA
                for i in range(13):
                    ps = proj_ps.tile([128, 512], F32, tag="proj", name="warm")
                    nc.tensor.matmul(ps, lhsT=dummy[:, 0:128], rhs=dummy,
                                     start=True, stop=True)

                # -- emitters --
                def emit_qk_group(dst, w_s, x_s, b_s, hp, nch):
                    ps = proj_ps.tile([128, 512], F32, tag="proj", name="qkp")
                    for t in range(DT):
                        nc.tensor.matmul(
                            ps,
                            lhsT=wview(w_s)[:, t, hp * 128:(hp + 1) * 128],
                            rhs=cview(x_s)[:, nch, t, :],
                            start=(t == 0),
                            stop=(t == DT - 1),
                        )
                    nc.vector.tensor_scalar_add(
                        dst[hp][:, nch * 512:(nch + 1) * 512], ps, b_s[hp]
                    )

                def emit_v_group(t):
                    ps = proj_ps.tile([128, EC], F32, tag="proj", name="vp")
                    for d in range(DT):
                        nc.tensor.matmul(
                            ps,
                            lhsT=cview(xv_sb)[:, t // 4, d,
                                              (t % 4) * 128:(t % 4 + 1) * 128],
                            rhs=wview(wv_sb)[:, d, :],
                            start=(d == 0),
                            stop=(d == DT - 1),
                        )
                    v_view = v_sb[t].rearrange("p (h c) -> p h c", c=65)[:, :, 0:64]
                    nc.vector.tensor_add(
                        v_view,
                        ps.rearrange("p (h c) -> p h c", c=64),
                        bvr_sb.rearrange("p (h c) -> p h c", c=64),
                    )

                def emit_filler(f):
                    if f[0] == "v":
                        emit_v_group(f[1])
                    elif f[0] == "q":
                        emit_qk_group(qt_sb, wq_sb, xq_sb, bq_sb, f[1], f[2])
                    else:
                        emit_qk_group(kt_sb, wk_sb, xk_sb, bk_sb, f[1], f[2])

                def emit_s_pair(hp, icol, j):
                    st = st_ps.tile([128, 1024], F32, tag="st", name="st")
                    for half in range(2):
                        ho = half * 64
                        nc.tensor.matmul(
                            st[:, half * 512:(half + 1) * 512],
                            lhsT=kt_sb[hp][ho:ho + 64, j * 128:(j + 1) * 128],
                            rhs=qt_sb[hp][ho:ho + 64, icol:icol + 512],
                            start=True,
                            stop=True,
                        )
                    return st

                ob_state = {}

                def emit_out_tr(hd, c, t):
                    # one transpose step of head hd's OT chunk c
                    if t == 0:
                        ob_state[(hd, c)] = proj_ps.tile(
                            [128, 512], F32, tag="proj", name="tr"
                        )
                    tr = ob_state[(hd, c)]
                    nc.tensor.transpose(
                        tr[:, t * 65:(t + 1) * 65],
                        ots_sb[hd][:, (c * 4 + t) * 128:(c * 4 + t + 1) * 128],
                        ident,
                    )

                def emit_out_fin(hd, c):
                    # batched recip, 4 muls, one DMA for the finished group
                    tr = ob_state.pop((hd, c))
                    tr3 = tr[:, 0:260].rearrange("p (t c) -> p t c", c=65)
                    rec = fin_pool.tile([128, 4], F32, tag="rec", name="rec")
                    nc.vector.reciprocal(rec, tr3[:, :, 64])
                    otile = fin_pool.tile([128, 256], F32, tag="otile", name="otile")
                    nc.vector.tensor_tensor(
                        otile.rearrange("p (t c) -> p t c", c=64),
                        tr3[:, :, 0:64],
                        rec.rearrange("p (t o) -> p t o", o=1).broadcast_to([128, 4, 64]),
                        mybir.AluOpType.mult,
                    )
                    blk = (hd * 4 + c) * 128
                    eng = nc.sync if (hd + c) % 2 == 0 else nc.gpsimd
                    eng.dma_start(out[blk:blk + 128, :], otile)

                # filler schedule: (pass, iter) -> list of jobs
                fillers = {
                    (0, 0): [("v", 0)], (0, 1): [("k", 0, 1), ("v", 1)],
                    (0, 2): [("v", 2), ("v", 3)], (0, 3): [("v", 4)],
                    (0, 4): [("v", 5), ("v", 6)], (0, 5): [("k", 0, 2)],
                    (0, 6): [("v", 7), ("v", 8)], (0, 7): [("v", 9)],
                    (0, 8): [("v", 10), ("v", 11)], (0, 9): [("k", 0, 3)],
                    (0, 10): [("v", 12), ("v", 13)],
                    (0, 11): [("q", 0, 1)],
                    (0, 12): [("v", 14)], (0, 13): [("v", 15)],
                    (1, 7): [("q", 0, 2)], (1, 13): [("k", 1, 0)],
                    (1, 14): [("k", 1, 1)],
                    (2, 7): [("q", 0, 3)], (2, 13): [("k", 1, 2)],
                    (2, 14): [("k", 1, 3)],
                    (3, 7): [("q", 1, 0)],
                    (4, 7): [("q", 1, 1)],
                    (5, 7): [("q", 1, 2)],
                    (6, 7): [("q", 1, 3)],
                }
                # out-block steps: (pass, iter) -> ("tr", head, chunk, t)
                # or ("fin", head, chunk); transposes spread one per iter
                obsteps = {}
                for p in range(1, 8):
                    hp_prev, c_prev = (p - 1) // 4, (p - 1) % 4
                    for t in range(4):
                        obsteps[(p, 2 + t)] = ("tr", 2 * hp_prev, c_prev, t)
                        obsteps[(p, 8 + t)] = ("tr", 2 * hp_prev + 1, c_prev, t)
                    obsteps[(p, 6)] = ("fin", 2 * hp_prev, c_prev)
                    obsteps[(p, 12)] = ("fin", 2 * hp_prev + 1, c_prev)

                # -- prologue for pass 0 --
                emit_qk_group(qt_sb, wq_sb, xq_sb, bq_sb, 0, 0)
                emit_qk_group(kt_sb, wk_sb, xk_sb, bk_sb, 0, 0)

                # wave 3 inputs: WAW-gated on the first QT chunk
                qg = qt_sb[0][0:1, 0:2]
                for sb, cc in ((xk_sb, 2), (xv_sb, 1), (xk_sb, 3),
                               (xq_sb, 1), (xv_sb, 2), (xq_sb, 2),
                               (xv_sb, 3), (xq_sb, 3)):
                    nc.vector.tensor_copy(
                        sb[0:1, cc * 2048:cc * 2048 + 2], qg)
                nc.gpsimd.dma_start(*xv_c(1))
                nc.gpsimd.dma_start(*xk_c(2))
                nc.gpsimd.dma_start(*xv_c(2))
                nc.gpsimd.dma_start(*xk_c(3))
                nc.gpsimd.dma_start(*xv_c(3))
                nc.gpsimd.dma_start(*xq_c(1))
                nc.gpsimd.dma_start(*xq_c(2))
                nc.gpsimd.dma_start(*xq_c(3))

                prologue = [None, None]
                for p in range(8):
                    hp, c = p // 4, p % 4
                    icol = c * 512
                    ha, hb = 2 * hp, 2 * hp + 1
                    sch = SCH_BY_PASS[p]
                    ot = ot_ps.tile([65, 1024], F32, tag="ot", name="ot")
                    sts = [None] * NT
                    pts = [None] * NT

                    if p == 0:
                        sts[0] = emit_s_pair(hp, icol, 0)
                        sts[1] = emit_s_pair(hp, icol, 1)
                    else:
                        sts[0], sts[1] = prologue

                    def emit_av_pair(j):
                        for half, hd in ((0, ha), (1, hb)):
                            nc.tensor.matmul(
                                ot[:, half * 512:(half + 1) * 512],
                                lhsT=v_sb[j][:, hd * 65:(hd + 1) * 65],
                                rhs=pts[j][:, half * 512:(half + 1) * 512],
                                start=(j == 0),
                                stop=(j == NT - 1),
                            )

                    def emit_sch(j):
                        # Schraudolph exp on DVE, one iter ahead of its slot
                        # so the st-buffer WAR never stalls the S pipeline
                        pti = pti_pool.tile([128, 1024], I16, tag="pti", name="pti")
                        nc.vector.tensor_scalar(
                            pti, sts[j], A_SCH, B_SCH,
                            mybir.AluOpType.mult, mybir.AluOpType.add,
                        )
                        pts[j] = pti.bitcast(BF16)

                    if 0 in sch:
                        emit_sch(0)
                    for j in range(NT):
                        if j not in sch:
                            pt = pt_pool.tile([128, 1024], BF16, tag="pt", name="pt")
                            nc.scalar.activation(
                                pt, sts[j], mybir.ActivationFunctionType.Exp,
                                scale=SCALE,
                            )
                            pts[j] = pt
                        if j + 1 in sch:
                            emit_sch(j + 1)
                        if j >= 2:
                            emit_av_pair(j - 2)
                        if j % 2 == 0:
                            for jj in (j + 2, j + 3):
                                if jj < NT:
                                    sts[jj] = emit_s_pair(hp, icol, jj)
                                elif p + 1 < 8:
                                    nhp, nc_ = (p + 1) // 4, (p + 1) % 4
                                    prologue[jj - NT] = emit_s_pair(
                                        nhp, nc_ * 512, jj - NT)
                        for f in fillers.get((p, j), ()):
                            emit_filler(f)
                        step = obsteps.get((p, j))
                        if step is not None:
                            if step[0] == "tr":
                                emit_out_tr(step[1], step[2], step[3])
                            else:
                                emit_out_fin(step[1], step[2])

                    emit_av_pair(NT - 2)
                    emit_av_pair(NT - 1)
                    nc.vector.tensor_copy(ots_sb[ha][:, icol:icol + 512], ot[:, 0:512])
                    nc.vector.tensor_copy(ots_sb[hb][:, icol:icol + 512], ot[:, 512:1024])

                # tail: last pair's final chunk
                for t in range(4):
                    emit_out_tr(2, 3, t)
                emit_out_fin(2, 3)
                for t in range(4):
                    emit_out_tr(3, 3, t)
                emit_out_fin(3, 3)

    nc.compile()
    return nc


def _get_nc():
    if "nc" not in _cache:
        _cache["nc"] = _build()
    return _cache["nc"]


def _ilv_x(xT):
    # [D, N] -> [128, 4c * 4t * 512n'] with row p = [c][t][n'] interleave
    return np.ascontiguousarray(
        xT.reshape(DT, 128, 4, 512).transpose(1, 2, 0, 3).reshape(128, DT * N)
    ).astype(NP_BF16)


def _ilv_w(w):
    # [D, EC] -> [128, 4t * EC]
    return np.ascontiguousarray(
        w.reshape(DT, 128, EC).transpose(1, 0, 2).reshape(128, DT * EC)
    ).astype(NP_BF16)


def _shard_inputs(q, k, v, Wq, Wk, Wv, bq, bk, bv):
    in_maps = []
    for c in range(8):
        b, g = c // 2, c % 2
        sl = slice(g * EC, (g + 1) * EC)
        in_maps.append({
            "xq": _ilv_x(np.asarray(q)[b].T),
            "xk": _ilv_x(np.asarray(k)[b].T),
            "xv": _ilv_x(np.asarray(v)[b].T),
            "wq": _ilv_w(np.asarray(Wq)[:, sl]),
            "wk": _ilv_w(np.asarray(Wk)[:, sl]),
            "wv": _ilv_w(np.asarray(Wv)[:, sl]),
            "bmat": np.stack([
                np.asarray(bq)[sl][0:128], np.asarray(bq)[sl][128:256],
                np.asarray(bk)[sl][0:128], np.asarray(bk)[sl][128:256],
            ], axis=1).astype(np.float32),
            "bvr": np.ascontiguousarray(
                np.broadcast_to(np.asarray(bv)[sl], (128, EC))
            ).astype(np.float32),
        })
    return in_maps


def kernel(q, k, v, Wq, Wk, Wv, bq, bk, bv, _trace=False):
    nc = _get_nc()
    in_maps = _shard_inputs(q, k, v, Wq, Wk, Wv, bq, bk, bv)
    res = run_bass_kernel_spmd(
        nc, in_maps, core_ids=list(range(8)), trace=_trace
    )
    out = np.empty((B, N, E), np.float32)
    for c in range(8):
        b, g = c // 2, c % 2
        o2 = res.results[c]["out"].reshape(HPC, 4, 128, 4, 64)
        out[b, :, g * EC:(g + 1) * EC] = (
            o2.transpose(1, 3, 2, 0, 4).reshape(N, EC)
        )
    if _trace:
        _cache["last_exec_time_ns"] = res.exec_time_ns
    return out


# revision 34
# speedup vs baseline: 1.0344x; 1.0071x over previous
"""Multi-head attention TRN2 Bass kernel (v2).

Problem: B=4, N=2048, D=E=512, 8 heads (ch=64).
out = softmax((x_q Wq + bq)(x_k Wk + bk)^T / 8) (x_v Wv + bv), per head.

Sharding (8 cores): core c handles batch b = c//2 and head-group g = c%2
(4 heads = 256 E-columns). Each core is fully independent (no collectives).

v2 changes over the original ACT-paced design:
  - Pass = (head-pair, i-chunk of 512). The two heads of a pair occupy
    SBUF partitions 0-63 / 64-127 of QT/KT, so their S^T matmuls issue as
    back-to-back row-tiled pairs (tile_position (0,0)/(64,0)) that execute
    CONCURRENTLY on the PE (HW-probed: 113 ns/MM vs 215 serial, 1.9x).
  - Part of the exp work moves off the ACT engine onto the DVE as a
    Schraudolph bit-trick: P_bf16bits = int16(rint(A*S + B)), one
    tensor_scalar (fp32 PSUM -> int16 SBUF, round-to-nearest verified on
    HW), bitcast to bf16 for the AV matmul. Host-simulated rel-err with
    this split: ~0.012 (gate 0.02).
  - Input DMAs are merged into few large 3D descriptors, spread across
    sync/vector/scalar/gpsimd queues, issued critical-first (wq+xq first).
  - PE warm-up dummies + early exp-table preload hide the HAM cold clock
    (4/8 = 1.2 GHz) and the 2.7us ACT table load during the input DMA.
  - Output blocks are staged 4-at-a-time in SBUF and written with one DMA
    per (head, 512-chunk): 16 output DMAs instead of 64.
"""

import numpy as np
import ml_dtypes

import concourse.bacc as bacc
import concourse.mybir as mybir
import concourse.tile as tile
from concourse.bass_utils import run_bass_kernel_spmd
from concourse.masks import make_identity

B, N, D, E = 4, 2048, 512, 512
H, CH = 8, 64
HPC = 4              # heads per core
EC = HPC * CH        # 256 E-columns per core
SCALE = 1.0 / 8.0    # 1/sqrt(CH)
NT = N // 128        # 16 j-tiles
DT = D // 128        # 4 d-tiles

SIGMA = 0.055
A_SCH = float(np.float32(128.0 * np.log2(np.e) * SCALE))
B_SCH = float(np.float32(128.0 * (127.0 - SIGMA)))
# Schraudolph j-tiles per pass (none in pass 0: it is projection-bound;
# j=15 keeps the pass-boundary st WAR off the last ACT)
SCH_BY_PASS = [()] + [(2, 5, 9, 12)] * 7

F32 = mybir.dt.float32
BF16 = mybir.dt.bfloat16
I16 = mybir.dt.int16
NP_BF16 = ml_dtypes.bfloat16

_cache = {}


def _build():
    nc = bacc.Bacc("TRN2", target_bir_lowering=False, debug=False)

    # x tensors host-interleaved: row p = [c-major][t-major][n'] so a
    # 512-column chunk is 4KB contiguous per row (DMA packets are
    # overhead-bound, so line size is the bandwidth lever)
    xq = nc.dram_tensor("xq", [128, DT * N], BF16, kind="ExternalInput")
    xk = nc.dram_tensor("xk", [128, DT * N], BF16, kind="ExternalInput")
    xv = nc.dram_tensor("xv", [128, DT * N], BF16, kind="ExternalInput")
    wq = nc.dram_tensor("wq", [128, DT * EC], BF16, kind="ExternalInput")
    wk = nc.dram_tensor("wk", [128, DT * EC], BF16, kind="ExternalInput")
    wv = nc.dram_tensor("wv", [128, DT * EC], BF16, kind="ExternalInput")
    # all four bias vectors as one [128, 4] tensor (cols: bq m0, bq m1,
    # bk m0, bk m1) -- a [128,1] DMA degenerates to 4-byte packets
    bmat = nc.dram_tensor("bmat", [128, 4], F32, kind="ExternalInput")
    bvr = nc.dram_tensor("bvr", [128, EC], F32, kind="ExternalInput")
    # output blocks land contiguous per (head, chunk); host reassembles
    out = nc.dram_tensor("out", [HPC * 4 * 128, 256], F32, kind="ExternalOutput")

    with tile.TileContext(nc) as tc:
        with (
            tc.tile_pool(name="singles", bufs=1) as singles,
            tc.tile_pool(name="qkv", bufs=1) as qkv,
            tc.tile_pool(name="fin", bufs=3) as fin_pool,
        ):
            # ---- SBUF staging ----
            dummy = singles.tile([128, 512], BF16, tag="dummy", name="dummy")
            # flat, chunk-major (c, t, n') so every chunk DMA is a 2D copy
            # with 4KB contiguous per partition (max DMA packet size)
            xq_sb = singles.tile([128, DT * N], BF16, tag="xq", name="xq")
            xk_sb = singles.tile([128, DT * N], BF16, tag="xk", name="xk")
            xv_sb = singles.tile([128, DT * N], BF16, tag="xv", name="xv")
            wq_sb = singles.tile([128, DT * EC], BF16, tag="wq", name="wq")
            wk_sb = singles.tile([128, DT * EC], BF16, tag="wk", name="wk")
            wv_sb = singles.tile([128, DT * EC], BF16, tag="wv", name="wv")
            bm_sb = singles.tile([128, 4], F32, tag="bm", name="bm")
            bq_sb = [bm_sb[:, m:m + 1] for m in range(2)]
            bk_sb = [bm_sb[:, 2 + m:3 + m] for m in range(2)]
            bvr_sb = singles.tile([128, EC], F32, tag="bvr", name="bvr")
            ident = singles.tile([65, 65], F32, tag="ident", name="ident")

            # ---- engine warm-up (emitted first on their queues) ----
            nc.vector.memset(dummy, 0.0)
            gate_sb = singles.tile([1, 8], BF16, tag="gate", name="gate")

            # ---- input DMAs: merged descriptors, critical-first ----
            def xq_c(c):
                return (xq_sb[:, c * 2048:(c + 1) * 2048],
                        xq[:, c * 2048:(c + 1) * 2048])

            def xk_c(c):
                return (xk_sb[:, c * 2048:(c + 1) * 2048],
                        xk[:, c * 2048:(c + 1) * 2048])

            def xv_c(c):
                return (xv_sb[:, c * 2048:(c + 1) * 2048],
                        xv[:, c * 2048:(c + 1) * 2048])

            def cview(sb):  # [128, 4c*4t*512] -> [128, c, t, n']
                return sb.rearrange("p (c t n) -> p c t n", c=4, t=DT)

            def wview(sb):  # [128, 4t*EC] -> [128, t, e]
                return sb.rearrange("p (t e) -> p t e", t=DT)

            # wave 1 (ungated): QK projection critical path + V pass-0 needs.
            # Everything else is gated behind wave-1 arrival by gpsimd
            # *compute* ops (a DMA's sem-wait rides the descriptor, so a
            # gating DMA would not block later queue entries -- a tensor_copy
            # does).
            nc.sync.dma_start(wq_sb, wq[:, :])
            nc.sync.dma_start(*xq_c(0))
            nc.scalar.dma_start(bm_sb, bmat[:, :])
            nc.scalar.dma_start(wk_sb, wk[:, :])
            nc.scalar.dma_start(*xk_c(0))
            # V wave gated on xk c0 so the K critical path gets full DMA bw
            nc.vector.tensor_copy(wv_sb[0:1, 0:2], xk_sb[0:1, 0:2])
            nc.vector.tensor_copy(xv_sb[0:1, 0:2], xk_sb[0:1, 0:2])
            nc.vector.tensor_copy(bvr_sb[0:1, 0:2], xk_sb[0:1, 0:4].bitcast(F32))
            nc.gpsimd.dma_start(wv_sb, wv[:, :])
            nc.gpsimd.dma_start(*xv_c(0))
            nc.gpsimd.dma_start(bvr_sb, bvr[:, :])
            # ACT table preload, after scalar's DMA issues
            actwarm = singles.tile([1, 8], BF16, tag="actwarm", name="actwarm")
            nc.scalar.activation(
                actwarm, dummy[0:1, 0:8], mybir.ActivationFunctionType.Exp,
                scale=SCALE,
            )
            # wave 2: WAW-gated on xq c0 arrival (tiny pre-write into the
            # DMA destination forces the DMA to wait; emission-order gating
            # does not survive the scheduler)
            nc.vector.tensor_copy(xk_sb[0:1, 2048:2050], xq_sb[0:1, 0:2])
            nc.gpsimd.dma_start(*xk_c(1))

            # ---- working tiles ----
            qt_sb = [qkv.tile([128, N], BF16, tag=f"qt{m}", name=f"qt{m}") for m in range(2)]
            kt_sb = [qkv.tile([128, N], BF16, tag=f"kt{m}", name=f"kt{m}") for m in range(2)]
            v_sb = [qkv.tile([128, HPC * 65], BF16, tag=f"v{t}", name=f"v{t}") for t in range(NT)]
            for t in range(NT):
                ones_view = v_sb[t].rearrange("p (h c) -> p h c", c=65)[:, :, 64:65]
                nc.vector.memset(ones_view, 1.0)
            ots_sb = [qkv.tile([65, N], F32, tag=f"ots{h}", name=f"ots{h}") for h in range(HPC)]
            make_identity(nc, ident)

            with (
                tc.tile_pool(name="proj_ps", bufs=2, space="PSUM") as proj_ps,
                tc.tile_pool(name="st_ps", bufs=2, space="PSUM") as st_ps,
                tc.tile_pool(name="ot_ps", bufs=1, space="PSUM") as ot_ps,
                tc.tile_pool(name="pt_sb", bufs=6) as pt_pool,
                tc.tile_pool(name="pti_sb", bufs=4) as pti_pool,
            ):
                # PE warm-up: ~14 dummy matmuls flip HAM to 8/8 during DMA
                for i in range(13):
                    ps = proj_ps.tile([128, 512], F32, tag="proj", name="warm")
                    nc.tensor.matmul(ps, lhsT=dummy[:, 0:128], rhs=dummy,
                                     start=True, stop=True)

                # -- emitters --
                def emit_qk_group(dst, w_s, x_s, b_s, hp, nch):
                    ps = proj_ps.tile([128, 512], F32, tag="proj", name="qkp")
                    for t in range(DT):
                        nc.tensor.matmul(
                            ps,
                            lhsT=wview(w_s)[:, t, hp * 128:(hp + 1) * 128],
                            rhs=cview(x_s)[:, nch, t, :],
                            start=(t == 0),
                            stop=(t == DT - 1),
                        )
                    nc.vector.tensor_scalar_add(
                        dst[hp][:, nch * 512:(nch + 1) * 512], ps, b_s[hp]
                    )

                def emit_v_group(t):
                    ps = proj_ps.tile([128, EC], F32, tag="proj", name="vp")
                    for d in range(DT):
                        nc.tensor.matmul(
                            ps,
                            lhsT=cview(xv_sb)[:, t // 4, d,
                                              (t % 4) * 128:(t % 4 + 1) * 128],
                            rhs=wview(wv_sb)[:, d, :],
                            start=(d == 0),
                            stop=(d == DT - 1),
                        )
                    v_view = v_sb[t].rearrange("p (h c) -> p h c", c=65)[:, :, 0:64]
                    nc.vector.tensor_add(
                        v_view,
                        ps.rearrange("p (h c) -> p h c", c=64),
                        bvr_sb.rearrange("p (h c) -> p h c", c=64),
                    )

                def emit_filler(f):
                    if f[0] == "v":
                        emit_v_group(f[1])
                    elif f[0] == "q":
                        emit_qk_group(qt_sb, wq_sb, xq_sb, bq_sb, f[1], f[2])
                    else:
                        emit_qk_group(kt_sb, wk_sb, xk_sb, bk_sb, f[1], f[2])

                def emit_s_pair(hp, icol, j):
                    st = st_ps.tile([128, 1024], F32, tag="st", name="st")
                    for half in range(2):
                        ho = half * 64
                        nc.tensor.matmul(
                            st[:, half * 512:(half + 1) * 512],
                            lhsT=kt_sb[hp][ho:ho + 64, j * 128:(j + 1) * 128],
                            rhs=qt_sb[hp][ho:ho + 64, icol:icol + 512],
                            start=True,
                            stop=True,
                        )
                    return st

                ob_state = {}

                def emit_out_tr(hd, c, t):
                    # one transpose step of head hd's OT chunk c
                    if t == 0:
                        ob_state[(hd, c)] = proj_ps.tile(
                            [128, 512], F32, tag="proj", name="tr"
                        )
                    tr = ob_state[(hd, c)]
                    nc.tensor.transpose(
                        tr[:, t * 65:(t + 1) * 65],
                        ots_sb[hd][:, (c * 4 + t) * 128:(c * 4 + t + 1) * 128],
                        ident,
                    )

                def emit_out_fin(hd, c):
                    # batched recip, 4 muls, one DMA for the finished group
                    tr = ob_state.pop((hd, c))
                    tr3 = tr[:, 0:260].rearrange("p (t c) -> p t c", c=65)
                    rec = fin_pool.tile([128, 4], F32, tag="rec", name="rec")
                    nc.vector.reciprocal(rec, tr3[:, :, 64])
                    otile = fin_pool.tile([128, 256], F32, tag="otile", name="otile")
                    for t in range(4):
                        nc.vector.tensor_scalar_mul(
                            otile[:, t * 64:(t + 1) * 64],
                            tr3[:, t, 0:64],
                            rec[:, t:t + 1],
                        )
                    blk = (hd * 4 + c) * 128
                    eng = nc.sync if (hd + c) % 2 == 0 else nc.gpsimd
                    eng.dma_start(out[blk:blk + 128, :], otile)

                # filler schedule: (pass, iter) -> list of jobs
                fillers = {
                    (0, 0): [("v", 0)], (0, 1): [("k", 0, 1), ("v", 1)],
                    (0, 2): [("v", 2), ("v", 3)], (0, 3): [("v", 4)],
                    (0, 4): [("v", 5), ("v", 6)], (0, 5): [("k", 0, 2)],
                    (0, 6): [("v", 7), ("v", 8)], (0, 7): [("v", 9)],
                    (0, 8): [("v", 10), ("v", 11)], (0, 9): [("k", 0, 3)],
                    (0, 10): [("v", 12), ("v", 13)],
                    (0, 11): [("q", 0, 1)],
                    (0, 12): [("v", 14)], (0, 13): [("v", 15)],
                    (1, 7): [("q", 0, 2)], (1, 13): [("k", 1, 0)],
                    (1, 14): [("k", 1, 1)],
                    (2, 7): [("q", 0, 3)], (2, 13): [("k", 1, 2)],
                    (2, 14): [("k", 1, 3)],
                    (3, 7): [("q", 1, 0)],
                    (4, 7): [("q", 1, 1)],
                    (5, 7): [("q", 1, 2)],
                    (6, 7): [("q", 1, 3)],
                }
                # out-block steps: (pass, iter) -> ("tr", head, chunk, t)
                # or ("fin", head, chunk); transposes spread one per iter
                obsteps = {}
                for p in range(1, 8):
                    hp_prev, c_prev = (p - 1) // 4, (p - 1) % 4
                    for t in range(4):
                        obsteps[(p, 2 + t)] = ("tr", 2 * hp_prev, c_prev, t)
                        obsteps[(p, 8 + t)] = ("tr", 2 * hp_prev + 1, c_prev, t)
                    obsteps[(p, 6)] = ("fin", 2 * hp_prev, c_prev)
                    obsteps[(p, 12)] = ("fin", 2 * hp_prev + 1, c_prev)

                # -- prologue for pass 0 --
                emit_qk_group(qt_sb, wq_sb, xq_sb, bq_sb, 0, 0)
                emit_qk_group(kt_sb, wk_sb, xk_sb, bk_sb, 0, 0)

                # wave 3 inputs: WAW-gated on the first QT chunk
                qg = qt_sb[0][0:1, 0:2]
                for sb, cc in ((xk_sb, 2), (xv_sb, 1), (xk_sb, 3),
                               (xq_sb, 1), (xv_sb, 2), (xq_sb, 2),
                               (xv_sb, 3), (xq_sb, 3)):
                    nc.vector.tensor_copy(
                        sb[0:1, cc * 2048:cc * 2048 + 2], qg)
                nc.gpsimd.dma_start(*xv_c(1))
                nc.gpsimd.dma_start(*xk_c(2))
                nc.gpsimd.dma_start(*xv_c(2))
                nc.gpsimd.dma_start(*xk_c(3))
                nc.gpsimd.dma_start(*xv_c(3))
                nc.gpsimd.dma_start(*xq_c(1))
                nc.gpsimd.dma_start(*xq_c(2))
                nc.gpsimd.dma_start(*xq_c(3))

                prologue = [None, None]
                for p in range(8):
                    hp, c = p // 4, p % 4
                    icol = c * 512
                    ha, hb = 2 * hp, 2 * hp + 1
                    sch = SCH_BY_PASS[p]
                    ot = ot_ps.tile([65, 1024], F32, tag="ot", name="ot")
                    sts = [None] * NT
                    pts = [None] * NT

                    if p == 0:
                        sts[0] = emit_s_pair(hp, icol, 0)
                        sts[1] = emit_s_pair(hp, icol, 1)
                    else:
                        sts[0], sts[1] = prologue

                    def emit_av_pair(j):
                        for half, hd in ((0, ha), (1, hb)):
                            nc.tensor.matmul(
                                ot[:, half * 512:(half + 1) * 512],
                                lhsT=v_sb[j][:, hd * 65:(hd + 1) * 65],
                                rhs=pts[j][:, half * 512:(half + 1) * 512],
                                start=(j == 0),
                                stop=(j == NT - 1),
                            )

                    def emit_sch(j):
                        # Schraudolph exp on DVE, one iter ahead of its slot
                        # so the st-buffer WAR never stalls the S pipeline
                        pti = pti_pool.tile([128, 1024], I16, tag="pti", name="pti")
                        nc.vector.tensor_scalar(
                            pti, sts[j], A_SCH, B_SCH,
                            mybir.AluOpType.mult, mybir.AluOpType.add,
                        )
                        pts[j] = pti.bitcast(BF16)

                    if 0 in sch:
                        emit_sch(0)
                    for j in range(NT):
                        if j not in sch:
                            pt = pt_pool.tile([128, 1024], BF16, tag="pt", name="pt")
                            nc.scalar.activation(
                                pt, sts[j], mybir.ActivationFunctionType.Exp,
                                scale=SCALE,
                            )
                            pts[j] = pt
                        if j + 1 in sch:
                            emit_sch(j + 1)
                        if j >= 2:
                            emit_av_pair(j - 2)
                        if j % 2 == 0:
                            for jj in (j + 2, j + 3):
                                if jj < NT:
                                    sts[jj] = emit_s_pair(hp, icol, jj)
                                elif p + 1 < 8:
                                    nhp, nc_ = (p + 1) // 4, (p + 1) % 4
                                    prologue[jj - NT] = emit_s_pair(
                                        nhp, nc_ * 512, jj - NT)
                        for f in fillers.get((p, j), ()):
                            emit_filler(f)
                        step = obsteps.get((p, j))
                        if step is not None:
                            if step[0] == "tr":
                                emit_out_tr(step[1], step[2], step[3])
                            else:
                                emit_out_fin(step[1], step[2])

                    emit_av_pair(NT - 2)
                    emit_av_pair(NT - 1)
                    nc.vector.tensor_copy(ots_sb[ha][:, icol:icol + 512], ot[:, 0:512])
                    nc.vector.tensor_copy(ots_sb[hb][:, icol:icol + 512], ot[:, 512:1024])

                # tail: last pair's final chunk
                for t in range(4):
                    emit_out_tr(2, 3, t)
                emit_out_fin(2, 3)
                for t in range(4):
                    emit_out_tr(3, 3, t)
                emit_out_fin(3, 3)

    nc.compile()
    return nc


def _get_nc():
    if "nc" not in _cache:
        _cache["nc"] = _build()
    return _cache["nc"]


def _ilv_x(xT):
    # [D, N] -> [128, 4c * 4t * 512n'] with row p = [c][t][n'] interleave
    return np.ascontiguousarray(
        xT.reshape(DT, 128, 4, 512).transpose(1, 2, 0, 3).reshape(128, DT * N)
    ).astype(NP_BF16)


def _ilv_w(w):
    # [D, EC] -> [128, 4t * EC]
    return np.ascontiguousarray(
        w.reshape(DT, 128, EC).transpose(1, 0, 2).reshape(128, DT * EC)
    ).astype(NP_BF16)


def _shard_inputs(q, k, v, Wq, Wk, Wv, bq, bk, bv):
    in_maps = []
    for c in range(8):
        b, g = c // 2, c % 2
        sl = slice(g * EC, (g + 1) * EC)
        in_maps.append({
            "xq": _ilv_x(np.asarray(q)[b].T),
            "xk": _ilv_x(np.asarray(k)[b].T),
            "xv": _ilv_x(np.asarray(v)[b].T),
            "wq": _ilv_w(np.asarray(Wq)[:, sl]),
            "wk": _ilv_w(np.asarray(Wk)[:, sl]),
            "wv": _ilv_w(np.asarray(Wv)[:, sl]),
            "bmat": np.stack([
                np.asarray(bq)[sl][0:128], np.asarray(bq)[sl][128:256],
                np.asarray(bk)[sl][0:128], np.asarray(bk)[sl][128:256],
            ], axis=1).astype(np.float32),
            "bvr": np.ascontiguousarray(
                np.broadcast_to(np.asarray(bv)[sl], (128, EC))
            ).astype(np.float32),
        })
    return in_maps


def kernel(q, k, v, Wq, Wk, Wv, bq, bk, bv, _trace=False):
    nc = _get_nc()
    in_maps = _shard_inputs(q, k, v, Wq, Wk, Wv, bq, bk, bv)
    res = run_bass_kernel_spmd(
        nc, in_maps, core_ids=list(range(8)), trace=_trace
    )
    out = np.empty((B, N, E), np.float32)
    for c in range(8):
        b, g = c // 2, c % 2
        o2 = res.results[c]["out"].reshape(HPC, 4, 128, 4, 64)
        out[b, :, g * EC:(g + 1) * EC] = (
            o2.transpose(1, 3, 2, 0, 4).reshape(N, EC)
        )
    if _trace:
        _cache["last_exec_time_ns"] = res.exec_time_ns
    return out
